# revision 27
# baseline (speedup 1.0000x reference)
"""Trainium2 Bass kernel for nn_MultiHeadAttention_26482768347194.

Key algebraic fact: the reference applies softmax over a size-1 trailing
axis, so the attention score matrix is exactly all-ones.  The whole module
collapses (exactly, in real arithmetic) to

    xsum[b]   = sum_l x[b, l, :]                        # (D,)
    t[b]      = xsum[b] @ wv + L * bv                   # (H*D,)
    z[b]      = t[b] @ fc_w + fc_b                      # (D,)
    y[b,l,:]  = x[b,l,:] + z[b]
    out       = LayerNorm(y) * ln_g + ln_b              # over last dim

q/k/tanh/score inputs are mathematically dead.

Sharding: pure data-parallel over batch, one batch element per core,
weights replicated; cross-core collectives cost ~70us under this runtime
(launch-skew barrier) so each core runs fully independently.

v2 design (vs the 64us baseline): the kernel is DMA-stream-bound on the
replicated 8MB bf16 weight load, with a long unoverlapped tail.  Changes:
  * wv / fc_w ship as fp8 e3m4 scaled by 64 (4MB instead of 8MB); PE
    matmuls run fp8-weights x bf16-activations (PE upconverts operands
    independently).  The exact bias path c = (L*bv) @ fc_w + fc_b is
    precomputed in fp32 on the host (it is batch-independent), so only
    the batch-dependent xsum @ wv @ fc term sees quantization.  Measured
    end-to-end absmax rel err of the full rounding model: ~1.1e-2.
  * out ships as fp16 (1MB instead of 2MB fp32), upcast on the host.
  * xsum via DVE free-axis reduces of x.T (frees ~32 PE matmuls).
  * t and z are computed in column form throughout ([128,k] tiles):
    t cols <- wv chunks (lhsT, fp8) x xsumT cols; z cols <- fc chunks
    (lhsT, fp8) x t cols.  No transposes or single-partition row ops on
    the critical path; zc columns feed the x.zc dot products directly.
  * layernorm tail collapsed to ONE DVE pass per token tile:
      out = xg * rstd + PSUM,  PSUM = ones (x) b  +  rstd (x) zg
    built by a single K=2 PE outer-product per tile ([ones; rstd_t]
    stationary, [b; zc*g] moving); xg = (x - mean_x) * ln_g and the
    per-token x statistics are computed on the DVE during the weight
    stream.  var_y = var_x + (2/D) x.zc + mean(z^2) - mean(z)^2.
  * DMA: few fat triggers (xT, x, 8 weight blocks, 2 output halves),
    4KB contiguous per partition per weight block, ordered so the
    weight stream starts immediately behind xT.

This file is self-contained: shapes are hardcoded, no sibling imports.
"""

from contextlib import ExitStack

import numpy as np
import ml_dtypes

import concourse.bass as bass
import concourse.bacc as bacc
import concourse.mybir as mybir
import concourse.tile as tile
from concourse.bass_utils import run_bass_kernel_spmd

B, L, D, H = 8, 1024, 512, 8
HD = H * D          # 4096
P = 128             # partitions
NT = L // P         # 8 token tiles per core
KD = D // P         # 4 contraction chunks over d
NB = HD // 512      # 8 weight blocks (512 hd columns each)
EPS = 1e-5
N_CORES = 8
S = 64.0            # fp8 weight scale
INV_S2 = 1.0 / (S * S)

F32 = mybir.dt.float32
F16 = mybir.dt.float16
BF16 = mybir.dt.bfloat16
F8 = mybir.dt.float8e3
AF = mybir.ActivationFunctionType
ALU = mybir.AluOpType


def build_kernel():
    nc = bacc.Bacc("TRN2", target_bir_lowering=False, debug=False,
                   num_devices=N_CORES)

    # host-blocked layouts; every big DMA reads 4-8KB contiguous per
    # partition row:
    #   xT[p, c, l]     = x[l, c*128 + p]                  (1MB bf16)
    #   x[p, t, d]      = x[t*128 + p, d]                  (1MB bf16)
    #   wvfc[j, p, m]   = 4x512 wv cols + 4x512 fc rows    (4MB fp8)
    #     wv part c*512+m  = wv_q[c*128 + p, j*512 + m]
    #     fc part oc*512+d = fc_q[(4j+oc)*128 + p, d]
    #   gb8 rows 0-3 = ln_g.reshape(4,128), rows 4-7 = ln_b.reshape(4,128)
    #   cT[p, blk]      = c[blk*128 + p],  c = (L*bv) @ fc_w + fc_b
    xT_d = nc.dram_tensor("xT", [P, KD, L], BF16, kind="ExternalInput")
    x_d = nc.dram_tensor("x", [P, NT, D], BF16, kind="ExternalInput")
    wvfc_d = nc.dram_tensor("wvfc", [NB, P, 8, 512], F8, kind="ExternalInput")
    id_d = nc.dram_tensor("id128", [P, P], BF16, kind="ExternalInput")
    g_d = nc.dram_tensor("grow", [1, D], F32, kind="ExternalInput")
    b_d = nc.dram_tensor("brow", [1, D], F32, kind="ExternalInput")
    c_d = nc.dram_tensor("crow", [1, D], F32, kind="ExternalInput")
    out_d = nc.dram_tensor("out", [L, D], F16, kind="ExternalOutput")
    import os
    dbg = os.environ.get("KERNEL_DEBUG_TAPS") == "1"
    if dbg:
        dbg_xs = nc.dram_tensor("dbg_xs", [P, KD], F32, kind="ExternalOutput")
        dbg_tT = nc.dram_tensor("dbg_tT", [P, 4 * NB], F32,
                                kind="ExternalOutput")
        dbg_z4 = nc.dram_tensor("dbg_z4", [1, D], F32, kind="ExternalOutput")
        dbg_r8 = nc.dram_tensor("dbg_r8", [P, NT], F32, kind="ExternalOutput")
        dbg_zg = nc.dram_tensor("dbg_zg", [1, D], F32, kind="ExternalOutput")

    out_v = out_d.ap().rearrange("(t p) d -> p t d", p=P)        # [P, NT, D]

    with tile.TileContext(nc, pool_alloc_mode="queue") as tc, \
            ExitStack() as ctx:
        ctx.enter_context(nc.allow_low_precision(
            reason="bf16 accumulator feeds, validated end-to-end ~1.1e-2"))
        consts = ctx.enter_context(tc.tile_pool(name="consts", bufs=1))
        work = ctx.enter_context(tc.tile_pool(name="work", bufs=3))
        psum = ctx.enter_context(
            tc.tile_pool(name="psum", bufs=1, space=bass.MemorySpace.PSUM))

        # ---- tiny SBUF constants (no DMA) ------------------------------
        ones2 = consts.tile([1, P], F32)         # K=1 broadcast lhsT
        nc.gpsimd.memset(ones2[:], 1.0)
        id1 = consts.tile([1, 1], BF16)          # 1x1 identity (row->col)
        nc.gpsimd.memset(id1[:], 1.0)
        eps_t = consts.tile([P, 1], F32)
        nc.gpsimd.memset(eps_t[:], EPS)
        ones2b = consts.tile([1, P], BF16)       # bf16 K=1 lhsT for b row
        nc.gpsimd.memset(ones2b[:], 1.0)

        # ---- DMA program: xT first, weights right behind, x mid --------
        xT_t = consts.tile([P, KD, L], BF16)
        nc.sync.dma_start(xT_t[:], xT_d.ap())

        wf_tiles = []
        for j in range(NB):
            wf = consts.tile([P, 8, 512], F8, tag="wf")
            wf_tiles.append(wf)
        nc.sync.dma_start(wf_tiles[0][:], wvfc_d.ap()[0])

        id_t = consts.tile([P, P], BF16)
        nc.sync.dma_start(id_t[:], id_d.ap())
        g_t = consts.tile([1, D], F32)
        nc.sync.dma_start(g_t[:], g_d.ap())
        b_t = consts.tile([1, D], F32)
        nc.sync.dma_start(b_t[:], b_d.ap())
        c_t = consts.tile([1, D], F32)
        nc.sync.dma_start(c_t[:], c_d.ap())

        nc.sync.dma_start(wf_tiles[1][:], wvfc_d.ap()[1])
        nc.sync.dma_start(wf_tiles[2][:], wvfc_d.ap()[2])

        x_t = consts.tile([P, NT, D], BF16)
        nc.sync.dma_start(x_t[:], x_d.ap())

        for j in range(3, NB):
            nc.sync.dma_start(wf_tiles[j][:], wvfc_d.ap()[j])

        # ---- xsum columns on the DVE (from xT) -------------------------
        xs_f = consts.tile([P, KD], F32)
        for c in range(KD):
            nc.vector.tensor_reduce(xs_f[:, c:c + 1], xT_t[:, c, :],
                                    axis=mybir.AxisListType.X, op=ALU.add)
        xsT = consts.tile([P, KD], BF16)
        nc.vector.tensor_copy(xsT[:], xs_f[:])

        # ---- g broadcast to [128, 512] for the xg pass -----------------
        ps_gbc = psum.tile([P, D], F32, tag="bigbank", bufs=1)
        nc.tensor.matmul(ps_gbc[:], ones2[:], g_t[:], start=True, stop=True)
        g_bc = consts.tile([P, D], F32)
        nc.vector.tensor_copy(g_bc[:], ps_gbc[:])

        # ---- weight stream -------------------------------------------
        # Per 512-wide hd block j:
        #   trow_j [1,512] = sum_c xsT[:,c].T @ wv_chunk(c)   (4 fat MMs,
        #     the wv bytes stream through the PE as rhs at 1 col/cycle)
        #   Scalar copies trow_j to SBUF bf16; 4 PE transposes give the
        #   tT columns; one DVE copy lands them in tT.
        #   zrow += tT_col(o).T @ fc_chunk(o)                 (4 fat MMs)
        # z MMs run two blocks behind the t MMs so the PE never waits on
        # the Scalar/DVE copy chain.  zrow accumulates in a single psum
        # bank with one start/stop group.
        trows = consts.tile([1, 4 * NB * P], BF16)     # t row, bf16, S-scaled
        tT = consts.tile([P, 4 * NB], BF16)
        ps_zrow = psum.tile([1, D], F32, tag="zrow", bufs=1)

        def emit_tblock(j):
            wf = wf_tiles[j]
            ps_tr = psum.tile([1, 512], F32, tag="trow", bufs=2)
            for c in range(KD):
                nc.tensor.matmul(ps_tr[:], xsT[:, c:c + 1], wf[:, c, :],
                                 start=(c == 0), stop=(c == KD - 1))
            nc.scalar.activation(trows[0:1, j * 512:(j + 1) * 512], ps_tr[:],
                                 AF.Identity)

        def emit_tpose(j):
            ps_tp = psum.tile([P, 4, 2], BF16, tag="tpose", bufs=2)
            for oc in range(4):
                o = 4 * j + oc
                nc.tensor.transpose(ps_tp[:, oc, 0:1],
                                    trows[0:1, o * P:(o + 1) * P], id1[:])
            nc.vector.tensor_copy(tT[:, 4 * j:4 * j + 4], ps_tp[:, :, 0])

        def emit_zblock(j):
            wf = wf_tiles[j]
            for oc in range(4):
                o = 4 * j + oc
                nc.tensor.matmul(ps_zrow[:], tT[:, o:o + 1], wf[:, 4 + oc, :],
                                 start=(o == 0), stop=(o == 4 * NB - 1))

        for j in range(NB):
            emit_tblock(j)
            if j >= 1:
                emit_tpose(j - 1)
            if j >= 2:
                emit_zblock(j - 2)
        emit_tpose(NB - 1)
        emit_zblock(NB - 2)
        emit_zblock(NB - 1)

        # ---- per-token x statistics + xg during the stream -------------
        varx8 = consts.tile([P, NT], F32)
        xg_tiles = []
        for t in range(NT):
            s6 = work.tile([P, 6], F32, tag="s6")
            nc.vector.bn_stats(s6[:], x_t[:, t, :])
            mv = work.tile([P, 2], F32, tag="mv")
            nc.vector.bn_aggr(mv[:], s6[:])
            nc.vector.tensor_copy(varx8[:, t:t + 1], mv[:, 1:2])
            negmx = work.tile([P, 1], F32, tag="negmx")
            nc.vector.tensor_scalar_mul(negmx[:], mv[:, 0:1], -1.0)
            xg = work.tile([P, D], BF16, tag="xg", bufs=8)
            nc.vector.scalar_tensor_tensor(
                xg[:], x_t[:, t, :], negmx[:], g_bc[:],
                op0=ALU.add, op1=ALU.mult)
            xg_tiles.append(xg)

        # b row in bf16 for the tail outer product (early, off critical path)
        brow_bf = consts.tile([1, D], BF16)
        nc.vector.tensor_copy(brow_bf[:], b_t[:])

        # ---- z tail: zrow -> zc row + zc cols, variance pieces ---------
        zrow = consts.tile([1, D], F32)
        zsum = consts.tile([1, 1], F32)
        nc.vector.scalar_tensor_tensor(
            zrow[:], ps_zrow[:], INV_S2, c_t[:], op0=ALU.mult, op1=ALU.add,
            accum_out=zsum[:])
        negmz = consts.tile([1, 1], F32)
        nc.scalar.mul(negmz[:], zsum[:], -1.0 / D)
        zqs = consts.tile([1, 1], F32)
        zsqrow = work.tile([1, D], F32, tag="zsq")
        nc.vector.scalar_tensor_tensor(
            zsqrow[:], zrow[:], 1.0, zrow[:], op0=ALU.mult, op1=ALU.mult,
            accum_out=zqs[:])
        # mean(z^2) - mean(z)^2 path (parallel with the zc path)
        mzsq = consts.tile([1, 1], F32)
        nc.vector.tensor_mul(mzsq[:], negmz[:], negmz[:])
        negmzsq = consts.tile([1, 1], F32)
        nc.vector.tensor_scalar_mul(negmzsq[:], mzsq[:], -1.0)
        ezv = consts.tile([1, 1], F32)
        nc.vector.scalar_tensor_tensor(
            ezv[:], zqs[:], 1.0 / D, negmzsq[:], op0=ALU.mult, op1=ALU.add)
        ps_ez = psum.tile([P, 1], F32, tag="small", bufs=2)
        nc.tensor.matmul(ps_ez[:], ones2[:], ezv[:], start=True, stop=True)
        bias8 = consts.tile([P, 1], F32)
        nc.scalar.activation(bias8[:], ps_ez[:], AF.Identity, bias=eps_t[:],
                             scale=1.0)

        # zc row (bf16) and its columns for the dot products
        zc_row = consts.tile([1, D], BF16)
        nc.scalar.activation(zc_row[:], zrow[:], AF.Identity, bias=negmz[:])
        ps_zc = psum.tile([P, KD, 2], BF16, tag="tpose", bufs=2)
        for r in range(KD):
            nc.tensor.transpose(ps_zc[:, r, 0:1],
                                zc_row[0:1, r * P:(r + 1) * P], id1[:])
        zc4 = consts.tile([P, KD], BF16)
        nc.vector.tensor_copy(zc4[:], ps_zc[:, :, 0])

        # zg row from zc row
        zgrow = consts.tile([1, D], BF16)
        nc.vector.scalar_tensor_tensor(
            zgrow[:], zc_row[:], 1.0, g_t[:], op0=ALU.mult, op1=ALU.mult)

        # ---- x.zc dots on the PE, batched variance/rstd ----------------
        pd8 = psum.tile([P, NT], F32, tag="trow", bufs=2)
        for t in range(NT):
            for c in range(KD):
                nc.tensor.matmul(
                    pd8[:, t:t + 1],
                    xT_t[:, c, t * P:(t + 1) * P],
                    zc4[:, c:c + 1],
                    start=(c == 0), stop=(c == KD - 1))
        var8 = consts.tile([P, NT], F32)
        nc.vector.scalar_tensor_tensor(
            var8[:], pd8[:], 2.0 / D, varx8[:], op0=ALU.mult, op1=ALU.add)
        std8 = consts.tile([P, NT], F32)
        nc.scalar.activation(std8[:], var8[:], AF.Sqrt, bias=bias8[:])
        rstd8 = consts.tile([P, NT], BF16)
        nc.vector.reciprocal(rstd8[:], std8[:])

        if dbg:
            dxs = consts.tile([P, KD], F32)
            nc.vector.tensor_copy(dxs[:], xsT[:])
            nc.sync.dma_start(dbg_xs.ap(), dxs[:])
            dtT = consts.tile([P, 4 * NB], F32)
            nc.vector.tensor_copy(dtT[:], tT[:])
            nc.sync.dma_start(dbg_tT.ap(), dtT[:])
            dz4 = consts.tile([1, D], F32)
            nc.vector.tensor_copy(dz4[:], zrow[:])
            nc.sync.dma_start(dbg_z4.ap(), dz4[:])
            dr8 = consts.tile([P, NT], F32)
            nc.vector.tensor_copy(dr8[:], rstd8[:])
            nc.sync.dma_start(dbg_r8.ap(), dr8[:])
            dzg = consts.tile([1, D], F32)
            nc.vector.tensor_copy(dzg[:], zgrow[:])
            nc.sync.dma_start(dbg_zg.ap(), dzg[:])

        # rstd rows via one column transpose per tile (base partition 0)
        rstd_rows = []
        for t in range(NT):
            ps_st = psum.tile([1, P], BF16, tag="small", bufs=2)
            nc.tensor.transpose(ps_st[:], rstd8[:, t:t + 1], id_t[:])
            rrow = work.tile([1, P], BF16, tag="rrow", bufs=8)
            nc.vector.tensor_copy(rrow[:], ps_st[:])
            rstd_rows.append(rrow)

        # ---- final: two outer-product matmuls + one DVE pass per tile --
        obuf = consts.tile([P, NT, D], F16)
        for t in range(NT):
            ps_o = psum.tile([P, D], F32, tag="bigbank", bufs=1)
            nc.tensor.matmul(ps_o[:], ones2b[:], brow_bf[:],
                             start=True, stop=False)
            nc.tensor.matmul(ps_o[:], rstd_rows[t][:], zgrow[:],
                             start=False, stop=True)
            nc.vector.scalar_tensor_tensor(
                obuf[:, t, :], xg_tiles[t][:], rstd8[:, t:t + 1], ps_o[:],
                op0=ALU.mult, op1=ALU.add)
            if t == NT // 2 - 1:
                nc.sync.dma_start(out_v[:, 0:NT // 2, :],
                                  obuf[:, 0:NT // 2, :])
        nc.sync.dma_start(out_v[:, NT // 2:NT, :], obuf[:, NT // 2:NT, :])

    nc.compile()
    return nc


_NC_CACHE = None


def _get_nc():
    global _NC_CACHE
    if _NC_CACHE is None:
        _NC_CACHE = build_kernel()
    return _NC_CACHE


def _shard_inputs(inputs):
    bf = ml_dtypes.bfloat16
    f8 = ml_dtypes.float8_e3m4
    x = np.asarray(inputs["input"], dtype=np.float32)
    wv = np.asarray(inputs["wv"], dtype=np.float32)
    bv = np.asarray(inputs["bv"], dtype=np.float32)
    fc_w = np.asarray(inputs["fc_w"], dtype=np.float32)
    fc_b = np.asarray(inputs["fc_b"], dtype=np.float32)
    ln_g = np.asarray(inputs["ln_g"], dtype=np.float32)
    ln_b = np.asarray(inputs["ln_b"], dtype=np.float32)

    wv_q = (wv * S).astype(f8)
    fc_q = (fc_w * S).astype(f8)
    # wv part:  [j, p, c, m]  = wv_q[c*128 + p, j*512 + m]
    wv_bl = wv_q.reshape(KD, P, NB, 512).transpose(2, 1, 0, 3)
    # fc part:  [j, p, oc, d] = fc_q[(4j + oc)*128 + p, d]
    fc_bl = fc_q.reshape(NB, 4, P, 512).transpose(0, 2, 1, 3)
    wvfc = np.ascontiguousarray(
        np.concatenate([wv_bl, fc_bl], axis=2))               # [8,128,8,512]

    c_vec = (float(L) * bv) @ fc_w + fc_b                     # exact fp32
    crow = np.ascontiguousarray(c_vec[None, :])               # [1, 512]
    grow = np.ascontiguousarray(ln_g[None, :])
    brow = np.ascontiguousarray(ln_b[None, :])
    id128 = np.eye(P, dtype=np.float32).astype(bf)

    in_maps = []
    for i in range(N_CORES):
        xT_bl = np.ascontiguousarray(
            x[i].T.reshape(KD, P, L).transpose(1, 0, 2)).astype(bf)
        x_bl = np.ascontiguousarray(
            x[i].reshape(NT, P, D).transpose(1, 0, 2)).astype(bf)
        in_maps.append({
            "xT": xT_bl,
            "x": x_bl,
            "wvfc": wvfc.reshape(NB, P, 8, 512),
            "id128": id128,
            "grow": grow,
            "brow": brow,
            "crow": crow,
        })
    return in_maps


def kernel(**inputs) -> np.ndarray:
    nc = _get_nc()
    in_maps = _shard_inputs(inputs)
    res = run_bass_kernel_spmd(nc, in_maps, core_ids=list(range(N_CORES)))
    out = np.stack([res.results[i]["out"] for i in range(N_CORES)], axis=0)
    return out.astype(np.float32)


def _install_ntff_hook_shim():
    """Bridge trn_boot's ctypes NTFF profiler into antenv.axon_hooks,
    which bass_utils imports when trace=True under axon."""
    import sys
    import types
    try:
        from antenv.axon_hooks import get_axon_ntff_profile_hook  # noqa: F401
        return
    except ImportError:
        pass
    try:
        from trn_agent_boot.trn_boot import _ntff_profile_via_ctypes
        hook = _ntff_profile_via_ctypes("/opt/axon/libaxon_pjrt.so")
    except Exception:
        hook = None
    mod = types.ModuleType("antenv.axon_hooks")
    state = {"hook": hook}
    mod.get_axon_ntff_profile_hook = lambda: state["hook"]
    mod.set_axon_ntff_profile_hook = lambda h: state.update(hook=h)
    sys.modules["antenv.axon_hooks"] = mod
    import antenv
    antenv.axon_hooks = mod


def kernel_profiled(inputs, trace_cores=None):
    """Like kernel() but with trace=True; returns (out, BassKernelResults)."""
    _install_ntff_hook_shim()
    nc = _get_nc()
    in_maps = _shard_inputs(inputs)
    res = run_bass_kernel_spmd(
        nc, in_maps, core_ids=list(range(N_CORES)), trace=True,
        trace_cores=trace_cores if trace_cores is not None else [0])
    out = np.stack([res.results[i]["out"] for i in range(N_CORES)], axis=0)
    return out.astype(np.float32), res


if __name__ == "__main__":
    import sys
    if "--sim" in sys.argv:
        # quick single-core CoreSim check against the collapsed math
        from concourse.bass_interp import CoreSim
        rng = np.random.default_rng(0)
        x = rng.standard_normal((B, L, D), dtype=np.float32)
        wv = rng.standard_normal((D, HD), dtype=np.float32) * 0.025
        bv = rng.standard_normal(HD, dtype=np.float32) * 0.025
        fc_w = rng.standard_normal((HD, D), dtype=np.float32) * 0.009
        fc_b = rng.standard_normal(D, dtype=np.float32) * 0.015
        g = rng.standard_normal(D, dtype=np.float32) * 0.3 + 1.0
        b = rng.standard_normal(D, dtype=np.float32) * 0.1
        inputs = dict(input=x, wv=wv, bv=bv, fc_w=fc_w, fc_b=fc_b,
                      ln_g=g, ln_b=b)

        nc = _get_nc()
        in_maps = _shard_inputs(inputs)
        sim = CoreSim(nc, trace=False)
        for k, v in in_maps[0].items():
            sim.tensor(k)[:] = v
        sim.simulate()
        got = np.array(sim.tensor("out")).astype(np.float32)

        xsum = x[0].sum(0)
        z = (xsum @ wv + L * bv) @ fc_w + fc_b
        y = x[0] + z[None, :]
        mu = y.mean(-1, keepdims=True)
        var = y.var(-1, keepdims=True)
        want = (y - mu) / np.sqrt(var + EPS) * g + b
        err = np.abs(got - want).max() / np.abs(want).max()
        print("sim absmax rel err:", err)
        assert err < 2e-2, err
        print("SIM PASS")


# revision 28
# speedup vs baseline: 1.4980x; 1.4980x over previous
"""Trainium2 Bass kernel for nn_MultiHeadAttention_26482768347194.

Key algebraic fact: the reference applies softmax over a size-1 trailing
axis, so the attention score matrix is exactly all-ones.  The whole module
collapses (exactly, in real arithmetic) to

    xsum[b]   = sum_l x[b, l, :]                        # (D,)
    t[b]      = xsum[b] @ wv + L * bv                   # (H*D,)
    z[b]      = t[b] @ fc_w + fc_b                      # (D,)
    y[b,l,:]  = x[b,l,:] + z[b]
    out       = LayerNorm(y) * ln_g + ln_b              # over last dim

q/k/tanh/score inputs are mathematically dead.

Sharding: pure data-parallel over batch, one batch element per core,
weights replicated; cross-core collectives cost ~70us under this runtime
(launch-skew barrier) so each core runs fully independently.

v2 design (vs the 64us baseline): the kernel is DMA-stream-bound on the
replicated 8MB bf16 weight load, with a long unoverlapped tail.  Changes:
  * wv / fc_w ship as fp8 e3m4 scaled by 64 (4MB instead of 8MB); PE
    matmuls run fp8-weights x bf16-activations (PE upconverts operands
    independently).  The exact bias path c = (L*bv) @ fc_w + fc_b is
    precomputed in fp32 on the host (it is batch-independent), so only
    the batch-dependent xsum @ wv @ fc term sees quantization.  Measured
    end-to-end absmax rel err of the full rounding model: ~1.1e-2.
  * out ships as fp16 (1MB instead of 2MB fp32), upcast on the host.
  * xsum via DVE free-axis reduces of x.T (frees ~32 PE matmuls).
  * t and z are computed in column form throughout ([128,k] tiles):
    t cols <- wv chunks (lhsT, fp8) x xsumT cols; z cols <- fc chunks
    (lhsT, fp8) x t cols.  No transposes or single-partition row ops on
    the critical path; zc columns feed the x.zc dot products directly.
  * layernorm tail collapsed to ONE DVE pass per token tile:
      out = xg * rstd + PSUM,  PSUM = ones (x) b  +  rstd (x) zg
    built by a single K=2 PE outer-product per tile ([ones; rstd_t]
    stationary, [b; zc*g] moving); xg = (x - mean_x) * ln_g and the
    per-token x statistics are computed on the DVE during the weight
    stream.  var_y = var_x + (2/D) x.zc + mean(z^2) - mean(z)^2.
  * DMA: few fat triggers (xT, x, 8 weight blocks, 2 output halves),
    4KB contiguous per partition per weight block, ordered so the
    weight stream starts immediately behind xT.

This file is self-contained: shapes are hardcoded, no sibling imports.
"""

from contextlib import ExitStack

import numpy as np
import ml_dtypes

import concourse.bass as bass
import concourse.bacc as bacc
import concourse.mybir as mybir
import concourse.tile as tile
from concourse.bass_utils import run_bass_kernel_spmd

B, L, D, H = 8, 1024, 512, 8
HD = H * D          # 4096
P = 128             # partitions
NT = L // P         # 8 token tiles per core
KD = D // P         # 4 contraction chunks over d
NB = HD // 512      # 8 weight blocks (512 hd columns each)
EPS = 1e-5
N_CORES = 8
S = 64.0            # fp8 weight scale
INV_S2 = 1.0 / (S * S)

F32 = mybir.dt.float32
F16 = mybir.dt.float16
BF16 = mybir.dt.bfloat16
F8 = mybir.dt.float8e3
AF = mybir.ActivationFunctionType
ALU = mybir.AluOpType


def build_kernel():
    nc = bacc.Bacc("TRN2", target_bir_lowering=False, debug=False,
                   num_devices=N_CORES)

    # host-blocked layouts; every big DMA reads 4-8KB contiguous per
    # partition row:
    #   xT[p, c, l]     = x[l, c*128 + p]                  (1MB bf16)
    #   x[p, t, d]      = x[t*128 + p, d]                  (1MB bf16)
    #   wvfc[j, p, m]   = 4x512 wv cols + 4x512 fc rows    (4MB fp8)
    #     wv part c*512+m  = wv_q[c*128 + p, j*512 + m]
    #     fc part oc*512+d = fc_q[(4j+oc)*128 + p, d]
    #   gb8 rows 0-3 = ln_g.reshape(4,128), rows 4-7 = ln_b.reshape(4,128)
    #   cT[p, blk]      = c[blk*128 + p],  c = (L*bv) @ fc_w + fc_b
    xT_d = nc.dram_tensor("xT", [P, KD, L], BF16, kind="ExternalInput")
    x_d = nc.dram_tensor("x", [P, NT, D], BF16, kind="ExternalInput")
    wvfc_d = nc.dram_tensor("wvfc", [NB, P, 8, 512], F8, kind="ExternalInput")
    id_d = nc.dram_tensor("id128", [P, P], BF16, kind="ExternalInput")
    g_d = nc.dram_tensor("grow", [1, D], F32, kind="ExternalInput")
    b_d = nc.dram_tensor("brow", [1, D], F32, kind="ExternalInput")
    c_d = nc.dram_tensor("crow", [1, D], F32, kind="ExternalInput")
    out_d = nc.dram_tensor("out", [L, D], F16, kind="ExternalOutput")
    import os
    dbg = os.environ.get("KERNEL_DEBUG_TAPS") == "1"
    if dbg:
        dbg_xs = nc.dram_tensor("dbg_xs", [P, KD], F32, kind="ExternalOutput")
        dbg_tT = nc.dram_tensor("dbg_tT", [P, 4 * NB], F32,
                                kind="ExternalOutput")
        dbg_z4 = nc.dram_tensor("dbg_z4", [1, D], F32, kind="ExternalOutput")
        dbg_r8 = nc.dram_tensor("dbg_r8", [P, NT], F32, kind="ExternalOutput")
        dbg_zg = nc.dram_tensor("dbg_zg", [1, D], F32, kind="ExternalOutput")

    out_v = out_d.ap().rearrange("(t p) d -> p t d", p=P)        # [P, NT, D]

    with tile.TileContext(nc, pool_alloc_mode="queue") as tc, \
            ExitStack() as ctx:
        ctx.enter_context(nc.allow_low_precision(
            reason="bf16 accumulator feeds, validated end-to-end ~1.1e-2"))
        consts = ctx.enter_context(tc.tile_pool(name="consts", bufs=1))
        work = ctx.enter_context(tc.tile_pool(name="work", bufs=3))
        psum = ctx.enter_context(
            tc.tile_pool(name="psum", bufs=1, space=bass.MemorySpace.PSUM))

        # ---- tiny SBUF constants (no DMA) ------------------------------
        ones2 = consts.tile([1, P], F32)         # K=1 broadcast lhsT
        nc.gpsimd.memset(ones2[:], 1.0)
        id1 = consts.tile([1, 1], BF16)          # 1x1 identity (row->col)
        nc.gpsimd.memset(id1[:], 1.0)
        eps_t = consts.tile([P, 1], F32)
        nc.gpsimd.memset(eps_t[:], EPS)
        ones2b = consts.tile([1, P], BF16)       # bf16 K=1 lhsT for b row
        nc.gpsimd.memset(ones2b[:], 1.0)

        # ---- DMA program: xT first, weights right behind, x mid --------
        xT_t = consts.tile([P, KD, L], BF16)
        nc.sync.dma_start(xT_t[:], xT_d.ap())

        wf_tiles = []
        for j in range(NB):
            wf = consts.tile([P, 8, 512], F8, tag="wf", bufs=NB)
            wf_tiles.append(wf)
        nc.sync.dma_start(wf_tiles[0][:], wvfc_d.ap()[0])

        id_t = consts.tile([P, P], BF16)
        nc.sync.dma_start(id_t[:], id_d.ap())
        g_t = consts.tile([1, D], F32)
        nc.sync.dma_start(g_t[:], g_d.ap())
        b_t = consts.tile([1, D], F32)
        nc.sync.dma_start(b_t[:], b_d.ap())
        c_t = consts.tile([1, D], F32)
        nc.sync.dma_start(c_t[:], c_d.ap())

        nc.sync.dma_start(wf_tiles[1][:], wvfc_d.ap()[1])
        nc.sync.dma_start(wf_tiles[2][:], wvfc_d.ap()[2])

        x_t = consts.tile([P, NT, D], BF16)
        nc.sync.dma_start(x_t[:], x_d.ap())

        for j in range(3, NB):
            nc.sync.dma_start(wf_tiles[j][:], wvfc_d.ap()[j])

        # ---- xsum columns on the DVE (from xT) -------------------------
        xs_f = consts.tile([P, KD], F32)
        for c in range(KD):
            nc.vector.tensor_reduce(xs_f[:, c:c + 1], xT_t[:, c, :],
                                    axis=mybir.AxisListType.X, op=ALU.add)
        xsT = consts.tile([P, KD], BF16)
        nc.vector.tensor_copy(xsT[:], xs_f[:])

        # ---- g broadcast to [128, 512] for the xg pass -----------------
        ps_gbc = psum.tile([P, D], F32, tag="bigbank", bufs=1)
        nc.tensor.matmul(ps_gbc[:], ones2[:], g_t[:], start=True, stop=True)
        g_bc = consts.tile([P, D], F32)
        nc.vector.tensor_copy(g_bc[:], ps_gbc[:])

        # ---- weight stream -------------------------------------------
        # Per 512-wide hd block j:
        #   trow_j [1,512] = sum_c xsT[:,c].T @ wv_chunk(c)   (4 fat MMs,
        #     the wv bytes stream through the PE as rhs at 1 col/cycle)
        #   Scalar copies trow_j to SBUF bf16; 4 PE transposes give the
        #   tT columns; one DVE copy lands them in tT.
        #   zrow += tT_col(o).T @ fc_chunk(o)                 (4 fat MMs)
        # z MMs run two blocks behind the t MMs so the PE never waits on
        # the Scalar/DVE copy chain.  zrow accumulates in a single psum
        # bank with one start/stop group.
        trows = consts.tile([1, 4 * NB * P], BF16)     # t row, bf16, S-scaled
        tT = consts.tile([P, 4 * NB], BF16)
        ps_zrow = psum.tile([1, D], F32, tag="zrow", bufs=1)

        def emit_tblock(j):
            wf = wf_tiles[j]
            ps_tr = psum.tile([1, 512], F32, tag="trow", bufs=2)
            for c in range(KD):
                nc.tensor.matmul(ps_tr[:], xsT[:, c:c + 1], wf[:, c, :],
                                 start=(c == 0), stop=(c == KD - 1))
            nc.scalar.activation(trows[0:1, j * 512:(j + 1) * 512], ps_tr[:],
                                 AF.Identity)

        def emit_tpose(j):
            ps_tp = psum.tile([P, 4, 2], BF16, tag="tpose", bufs=2)
            for oc in range(4):
                o = 4 * j + oc
                nc.tensor.transpose(ps_tp[:, oc, 0:1],
                                    trows[0:1, o * P:(o + 1) * P], id1[:])
            nc.vector.tensor_copy(tT[:, 4 * j:4 * j + 4], ps_tp[:, :, 0])

        def emit_zblock(j):
            wf = wf_tiles[j]
            for oc in range(4):
                o = 4 * j + oc
                nc.tensor.matmul(ps_zrow[:], tT[:, o:o + 1], wf[:, 4 + oc, :],
                                 start=(o == 0), stop=(o == 4 * NB - 1))

        for j in range(NB):
            emit_tblock(j)
            if j >= 1:
                emit_tpose(j - 1)
            if j >= 2:
                emit_zblock(j - 2)
        emit_tpose(NB - 1)
        emit_zblock(NB - 2)
        emit_zblock(NB - 1)

        # ---- per-token x statistics + xg during the stream -------------
        varx8 = consts.tile([P, NT], F32)
        xg_tiles = []
        for t in range(NT):
            s6 = work.tile([P, 6], F32, tag="s6")
            nc.vector.bn_stats(s6[:], x_t[:, t, :])
            mv = work.tile([P, 2], F32, tag="mv")
            nc.vector.bn_aggr(mv[:], s6[:])
            nc.vector.tensor_copy(varx8[:, t:t + 1], mv[:, 1:2])
            negmx = work.tile([P, 1], F32, tag="negmx")
            nc.vector.tensor_scalar_mul(negmx[:], mv[:, 0:1], -1.0)
            xg = work.tile([P, D], BF16, tag="xg", bufs=8)
            nc.vector.scalar_tensor_tensor(
                xg[:], x_t[:, t, :], negmx[:], g_bc[:],
                op0=ALU.add, op1=ALU.mult)
            xg_tiles.append(xg)

        # b row in bf16 for the tail outer product (early, off critical path)
        brow_bf = consts.tile([1, D], BF16)
        nc.vector.tensor_copy(brow_bf[:], b_t[:])

        # ---- z tail: zrow -> zc row + zc cols, variance pieces ---------
        zrow = consts.tile([1, D], F32)
        zsum = consts.tile([1, 1], F32)
        nc.vector.scalar_tensor_tensor(
            zrow[:], ps_zrow[:], INV_S2, c_t[:], op0=ALU.mult, op1=ALU.add,
            accum_out=zsum[:])
        negmz = consts.tile([1, 1], F32)
        nc.scalar.mul(negmz[:], zsum[:], -1.0 / D)
        zqs = consts.tile([1, 1], F32)
        zsqrow = work.tile([1, D], F32, tag="zsq")
        nc.vector.scalar_tensor_tensor(
            zsqrow[:], zrow[:], 1.0, zrow[:], op0=ALU.mult, op1=ALU.mult,
            accum_out=zqs[:])
        # mean(z^2) - mean(z)^2 path (parallel with the zc path)
        mzsq = consts.tile([1, 1], F32)
        nc.vector.tensor_mul(mzsq[:], negmz[:], negmz[:])
        negmzsq = consts.tile([1, 1], F32)
        nc.vector.tensor_scalar_mul(negmzsq[:], mzsq[:], -1.0)
        ezv = consts.tile([1, 1], F32)
        nc.vector.scalar_tensor_tensor(
            ezv[:], zqs[:], 1.0 / D, negmzsq[:], op0=ALU.mult, op1=ALU.add)
        ps_ez = psum.tile([P, 1], F32, tag="small", bufs=2)
        nc.tensor.matmul(ps_ez[:], ones2[:], ezv[:], start=True, stop=True)
        bias8 = consts.tile([P, 1], F32)
        nc.scalar.activation(bias8[:], ps_ez[:], AF.Identity, bias=eps_t[:],
                             scale=1.0)

        # zc row (bf16) and its columns for the dot products
        zc_row = consts.tile([1, D], BF16)
        nc.scalar.activation(zc_row[:], zrow[:], AF.Identity, bias=negmz[:])
        ps_zc = psum.tile([P, KD, 2], BF16, tag="tpose", bufs=2)
        for r in range(KD):
            nc.tensor.transpose(ps_zc[:, r, 0:1],
                                zc_row[0:1, r * P:(r + 1) * P], id1[:])
        zc4 = consts.tile([P, KD], BF16)
        nc.vector.tensor_copy(zc4[:], ps_zc[:, :, 0])

        # zg row from zc row
        zgrow = consts.tile([1, D], BF16)
        nc.vector.scalar_tensor_tensor(
            zgrow[:], zc_row[:], 1.0, g_t[:], op0=ALU.mult, op1=ALU.mult)

        # ---- x.zc dots on the PE, batched variance/rstd ----------------
        pd8 = psum.tile([P, NT], F32, tag="trow", bufs=2)
        for t in range(NT):
            for c in range(KD):
                nc.tensor.matmul(
                    pd8[:, t:t + 1],
                    xT_t[:, c, t * P:(t + 1) * P],
                    zc4[:, c:c + 1],
                    start=(c == 0), stop=(c == KD - 1))
        var8 = consts.tile([P, NT], F32)
        nc.vector.scalar_tensor_tensor(
            var8[:], pd8[:], 2.0 / D, varx8[:], op0=ALU.mult, op1=ALU.add)
        std8 = consts.tile([P, NT], F32)
        nc.scalar.activation(std8[:], var8[:], AF.Sqrt, bias=bias8[:])
        rstd8 = consts.tile([P, NT], BF16)
        nc.vector.reciprocal(rstd8[:], std8[:])

        if dbg:
            dxs = consts.tile([P, KD], F32)
            nc.vector.tensor_copy(dxs[:], xsT[:])
            nc.sync.dma_start(dbg_xs.ap(), dxs[:])
            dtT = consts.tile([P, 4 * NB], F32)
            nc.vector.tensor_copy(dtT[:], tT[:])
            nc.sync.dma_start(dbg_tT.ap(), dtT[:])
            dz4 = consts.tile([1, D], F32)
            nc.vector.tensor_copy(dz4[:], zrow[:])
            nc.sync.dma_start(dbg_z4.ap(), dz4[:])
            dr8 = consts.tile([P, NT], F32)
            nc.vector.tensor_copy(dr8[:], rstd8[:])
            nc.sync.dma_start(dbg_r8.ap(), dr8[:])
            dzg = consts.tile([1, D], F32)
            nc.vector.tensor_copy(dzg[:], zgrow[:])
            nc.sync.dma_start(dbg_zg.ap(), dzg[:])

        # rstd rows via one column transpose per tile (base partition 0)
        rstd_rows = []
        for t in range(NT):
            ps_st = psum.tile([1, P], BF16, tag="small", bufs=2)
            nc.tensor.transpose(ps_st[:], rstd8[:, t:t + 1], id_t[:])
            rrow = work.tile([1, P], BF16, tag="rrow", bufs=8)
            nc.vector.tensor_copy(rrow[:], ps_st[:])
            rstd_rows.append(rrow)

        # ---- final: two outer-product matmuls + one DVE pass per tile --
        obuf = consts.tile([P, NT, D], F16)
        for t in range(NT):
            ps_o = psum.tile([P, D], F32, tag="bigbank", bufs=1)
            nc.tensor.matmul(ps_o[:], ones2b[:], brow_bf[:],
                             start=True, stop=False)
            nc.tensor.matmul(ps_o[:], rstd_rows[t][:], zgrow[:],
                             start=False, stop=True)
            nc.vector.scalar_tensor_tensor(
                obuf[:, t, :], xg_tiles[t][:], rstd8[:, t:t + 1], ps_o[:],
                op0=ALU.mult, op1=ALU.add)
            if t == NT // 2 - 1:
                nc.sync.dma_start(out_v[:, 0:NT // 2, :],
                                  obuf[:, 0:NT // 2, :])
        nc.sync.dma_start(out_v[:, NT // 2:NT, :], obuf[:, NT // 2:NT, :])

    nc.compile()
    return nc


_NC_CACHE = None


def _get_nc():
    global _NC_CACHE
    if _NC_CACHE is None:
        _NC_CACHE = build_kernel()
    return _NC_CACHE


def _shard_inputs(inputs):
    bf = ml_dtypes.bfloat16
    f8 = ml_dtypes.float8_e3m4
    x = np.asarray(inputs["input"], dtype=np.float32)
    wv = np.asarray(inputs["wv"], dtype=np.float32)
    bv = np.asarray(inputs["bv"], dtype=np.float32)
    fc_w = np.asarray(inputs["fc_w"], dtype=np.float32)
    fc_b = np.asarray(inputs["fc_b"], dtype=np.float32)
    ln_g = np.asarray(inputs["ln_g"], dtype=np.float32)
    ln_b = np.asarray(inputs["ln_b"], dtype=np.float32)

    wv_q = (wv * S).astype(f8)
    fc_q = (fc_w * S).astype(f8)
    # wv part:  [j, p, c, m]  = wv_q[c*128 + p, j*512 + m]
    wv_bl = wv_q.reshape(KD, P, NB, 512).transpose(2, 1, 0, 3)
    # fc part:  [j, p, oc, d] = fc_q[(4j + oc)*128 + p, d]
    fc_bl = fc_q.reshape(NB, 4, P, 512).transpose(0, 2, 1, 3)
    wvfc = np.ascontiguousarray(
        np.concatenate([wv_bl, fc_bl], axis=2))               # [8,128,8,512]

    c_vec = (float(L) * bv) @ fc_w + fc_b                     # exact fp32
    crow = np.ascontiguousarray(c_vec[None, :])               # [1, 512]
    grow = np.ascontiguousarray(ln_g[None, :])
    brow = np.ascontiguousarray(ln_b[None, :])
    id128 = np.eye(P, dtype=np.float32).astype(bf)

    in_maps = []
    for i in range(N_CORES):
        xT_bl = np.ascontiguousarray(
            x[i].T.reshape(KD, P, L).transpose(1, 0, 2)).astype(bf)
        x_bl = np.ascontiguousarray(
            x[i].reshape(NT, P, D).transpose(1, 0, 2)).astype(bf)
        in_maps.append({
            "xT": xT_bl,
            "x": x_bl,
            "wvfc": wvfc.reshape(NB, P, 8, 512),
            "id128": id128,
            "grow": grow,
            "brow": brow,
            "crow": crow,
        })
    return in_maps


def kernel(**inputs) -> np.ndarray:
    nc = _get_nc()
    in_maps = _shard_inputs(inputs)
    res = run_bass_kernel_spmd(nc, in_maps, core_ids=list(range(N_CORES)))
    out = np.stack([res.results[i]["out"] for i in range(N_CORES)], axis=0)
    return out.astype(np.float32)


def _install_ntff_hook_shim():
    """Bridge trn_boot's ctypes NTFF profiler into antenv.axon_hooks,
    which bass_utils imports when trace=True under axon."""
    import sys
    import types
    try:
        from antenv.axon_hooks import get_axon_ntff_profile_hook  # noqa: F401
        return
    except ImportError:
        pass
    try:
        from trn_agent_boot.trn_boot import _ntff_profile_via_ctypes
        hook = _ntff_profile_via_ctypes("/opt/axon/libaxon_pjrt.so")
    except Exception:
        hook = None
    mod = types.ModuleType("antenv.axon_hooks")
    state = {"hook": hook}
    mod.get_axon_ntff_profile_hook = lambda: state["hook"]
    mod.set_axon_ntff_profile_hook = lambda h: state.update(hook=h)
    sys.modules["antenv.axon_hooks"] = mod
    import antenv
    antenv.axon_hooks = mod


def kernel_profiled(inputs, trace_cores=None):
    """Like kernel() but with trace=True; returns (out, BassKernelResults)."""
    _install_ntff_hook_shim()
    nc = _get_nc()
    in_maps = _shard_inputs(inputs)
    res = run_bass_kernel_spmd(
        nc, in_maps, core_ids=list(range(N_CORES)), trace=True,
        trace_cores=trace_cores if trace_cores is not None else [0])
    out = np.stack([res.results[i]["out"] for i in range(N_CORES)], axis=0)
    return out.astype(np.float32), res


if __name__ == "__main__":
    import sys
    if "--sim" in sys.argv:
        # quick single-core CoreSim check against the collapsed math
        from concourse.bass_interp import CoreSim
        rng = np.random.default_rng(0)
        x = rng.standard_normal((B, L, D), dtype=np.float32)
        wv = rng.standard_normal((D, HD), dtype=np.float32) * 0.025
        bv = rng.standard_normal(HD, dtype=np.float32) * 0.025
        fc_w = rng.standard_normal((HD, D), dtype=np.float32) * 0.009
        fc_b = rng.standard_normal(D, dtype=np.float32) * 0.015
        g = rng.standard_normal(D, dtype=np.float32) * 0.3 + 1.0
        b = rng.standard_normal(D, dtype=np.float32) * 0.1
        inputs = dict(input=x, wv=wv, bv=bv, fc_w=fc_w, fc_b=fc_b,
                      ln_g=g, ln_b=b)

        nc = _get_nc()
        in_maps = _shard_inputs(inputs)
        sim = CoreSim(nc, trace=False)
        for k, v in in_maps[0].items():
            sim.tensor(k)[:] = v
        sim.simulate()
        got = np.array(sim.tensor("out")).astype(np.float32)

        xsum = x[0].sum(0)
        z = (xsum @ wv + L * bv) @ fc_w + fc_b
        y = x[0] + z[None, :]
        mu = y.mean(-1, keepdims=True)
        var = y.var(-1, keepdims=True)
        want = (y - mu) / np.sqrt(var + EPS) * g + b
        err = np.abs(got - want).max() / np.abs(want).max()
        print("sim absmax rel err:", err)
        assert err < 2e-2, err
        print("SIM PASS")


# revision 29
# speedup vs baseline: 1.8945x; 1.2647x over previous
"""Trainium2 Bass kernel for nn_MultiHeadAttention_26482768347194.

Key algebraic fact: the reference applies softmax over a size-1 trailing
axis, so the attention score matrix is exactly all-ones.  The whole module
collapses (exactly, in real arithmetic) to

    xsum[b]   = sum_l x[b, l, :]                        # (D,)
    t[b]      = xsum[b] @ wv + L * bv                   # (H*D,)
    z[b]      = t[b] @ fc_w + fc_b                      # (D,)
    y[b,l,:]  = x[b,l,:] + z[b]
    out       = LayerNorm(y) * ln_g + ln_b              # over last dim

q/k/tanh/score inputs are mathematically dead.

Sharding: pure data-parallel over batch, one batch element per core,
weights replicated; cross-core collectives cost ~70us under this runtime
(launch-skew barrier) so each core runs fully independently.

v2 design (vs the 64us baseline): the kernel is DMA-stream-bound on the
replicated 8MB bf16 weight load, with a long unoverlapped tail.  Changes:
  * wv / fc_w ship as fp8 e3m4 scaled by 64 (4MB instead of 8MB); PE
    matmuls run fp8-weights x bf16-activations (PE upconverts operands
    independently).  The exact bias path c = (L*bv) @ fc_w + fc_b is
    precomputed in fp32 on the host (it is batch-independent), so only
    the batch-dependent xsum @ wv @ fc term sees quantization.  Measured
    end-to-end absmax rel err of the full rounding model: ~1.1e-2.
  * out ships as fp16 (1MB instead of 2MB fp32), upcast on the host.
  * xsum via DVE free-axis reduces of x.T (frees ~32 PE matmuls).
  * t and z are computed in column form throughout ([128,k] tiles):
    t cols <- wv chunks (lhsT, fp8) x xsumT cols; z cols <- fc chunks
    (lhsT, fp8) x t cols.  No transposes or single-partition row ops on
    the critical path; zc columns feed the x.zc dot products directly.
  * layernorm tail collapsed to ONE DVE pass per token tile:
      out = xg * rstd + PSUM,  PSUM = ones (x) b  +  rstd (x) zg
    built by a single K=2 PE outer-product per tile ([ones; rstd_t]
    stationary, [b; zc*g] moving); xg = (x - mean_x) * ln_g and the
    per-token x statistics are computed on the DVE during the weight
    stream.  var_y = var_x + (2/D) x.zc + mean(z^2) - mean(z)^2.
  * DMA: few fat triggers (xT, x, 8 weight blocks, 2 output halves),
    4KB contiguous per partition per weight block, ordered so the
    weight stream starts immediately behind xT.

This file is self-contained: shapes are hardcoded, no sibling imports.
"""

from contextlib import ExitStack

import numpy as np
import ml_dtypes

import concourse.bass as bass
import concourse.bacc as bacc
import concourse.mybir as mybir
import concourse.tile as tile
from concourse.bass_utils import run_bass_kernel_spmd

B, L, D, H = 8, 1024, 512, 8
HD = H * D          # 4096
P = 128             # partitions
NT = L // P         # 8 token tiles per core
KD = D // P         # 4 contraction chunks over d
NB = HD // 512      # 8 weight blocks (512 hd columns each)
EPS = 1e-5
N_CORES = 8
S = 64.0            # fp8 weight scale
INV_S2 = 1.0 / (S * S)

F32 = mybir.dt.float32
F16 = mybir.dt.float16
BF16 = mybir.dt.bfloat16
F8 = mybir.dt.float8e3
AF = mybir.ActivationFunctionType
ALU = mybir.AluOpType


def build_kernel():
    nc = bacc.Bacc("TRN2", target_bir_lowering=False, debug=False,
                   num_devices=N_CORES)

    # host-blocked layouts; every big DMA reads 4-8KB contiguous per
    # partition row:
    #   xT[p, c, l]     = x[l, c*128 + p]                  (1MB bf16)
    #   x[p, t, d]      = x[t*128 + p, d]                  (1MB bf16)
    #   wvfc[j, p, m]   = 4x512 wv cols + 4x512 fc rows    (4MB fp8)
    #     wv part c*512+m  = wv_q[c*128 + p, j*512 + m]
    #     fc part oc*512+d = fc_q[(4j+oc)*128 + p, d]
    #   gb8 rows 0-3 = ln_g.reshape(4,128), rows 4-7 = ln_b.reshape(4,128)
    #   cT[p, blk]      = c[blk*128 + p],  c = (L*bv) @ fc_w + fc_b
    xT_d = nc.dram_tensor("xT", [P, KD, L], BF16, kind="ExternalInput")
    x_d = nc.dram_tensor("x", [P, NT, D], BF16, kind="ExternalInput")
    wvfc_d = nc.dram_tensor("wvfc", [NB, P, 8, 512], F8, kind="ExternalInput")
    id_d = nc.dram_tensor("id128", [P, P], BF16, kind="ExternalInput")
    g_d = nc.dram_tensor("grow", [1, D], F32, kind="ExternalInput")
    b_d = nc.dram_tensor("brow", [1, D], F32, kind="ExternalInput")
    c_d = nc.dram_tensor("crow", [1, D], F32, kind="ExternalInput")
    out_d = nc.dram_tensor("out", [L, D], F16, kind="ExternalOutput")
    import os
    dbg = os.environ.get("KERNEL_DEBUG_TAPS") == "1"
    if dbg:
        dbg_xs = nc.dram_tensor("dbg_xs", [P, KD], F32, kind="ExternalOutput")
        dbg_tT = nc.dram_tensor("dbg_tT", [P, 4 * NB], F32,
                                kind="ExternalOutput")
        dbg_z4 = nc.dram_tensor("dbg_z4", [1, D], F32, kind="ExternalOutput")
        dbg_r8 = nc.dram_tensor("dbg_r8", [P, NT], F32, kind="ExternalOutput")
        dbg_zg = nc.dram_tensor("dbg_zg", [1, D], F32, kind="ExternalOutput")

    out_v = out_d.ap().rearrange("(t p) d -> p t d", p=P)        # [P, NT, D]

    with tile.TileContext(nc, pool_alloc_mode="queue") as tc, \
            ExitStack() as ctx:
        ctx.enter_context(nc.allow_low_precision(
            reason="bf16 accumulator feeds, validated end-to-end ~1.1e-2"))
        consts = ctx.enter_context(tc.tile_pool(name="consts", bufs=1))
        work = ctx.enter_context(tc.tile_pool(name="work", bufs=3))
        psum = ctx.enter_context(
            tc.tile_pool(name="psum", bufs=1, space=bass.MemorySpace.PSUM))

        # ---- tiny SBUF constants (no DMA) ------------------------------
        ones2 = consts.tile([1, P], F32)         # K=1 broadcast lhsT
        nc.gpsimd.memset(ones2[:], 1.0)
        id1 = consts.tile([1, 1], BF16)          # 1x1 identity (row->col)
        nc.gpsimd.memset(id1[:], 1.0)
        eps_t = consts.tile([P, 1], F32)
        nc.gpsimd.memset(eps_t[:], EPS)
        ones2b = consts.tile([1, P], BF16)       # bf16 K=1 lhsT for b row
        nc.gpsimd.memset(ones2b[:], 1.0)

        # ---- DMA program: xT first, weights right behind, x mid --------
        xT_t = consts.tile([P, KD, L], BF16)
        nc.sync.dma_start(xT_t[:], xT_d.ap())

        wf_tiles = []
        for j in range(NB):
            wf = consts.tile([P, 8, 512], F8, tag="wf", bufs=NB)
            wf_tiles.append(wf)
        nc.sync.dma_start(wf_tiles[0][:], wvfc_d.ap()[0])

        id_t = consts.tile([P, P], BF16)
        nc.sync.dma_start(id_t[:], id_d.ap())
        g_t = consts.tile([1, D], F32)
        nc.sync.dma_start(g_t[:], g_d.ap())
        b_t = consts.tile([1, D], F32)
        nc.sync.dma_start(b_t[:], b_d.ap())
        c_t = consts.tile([1, D], F32)
        nc.sync.dma_start(c_t[:], c_d.ap())

        nc.sync.dma_start(wf_tiles[1][:], wvfc_d.ap()[1])
        nc.sync.dma_start(wf_tiles[2][:], wvfc_d.ap()[2])

        x_t = consts.tile([P, NT, D], BF16)
        nc.sync.dma_start(x_t[:], x_d.ap())

        for j in range(3, NB):
            nc.sync.dma_start(wf_tiles[j][:], wvfc_d.ap()[j])

        # ---- xsum columns on the DVE (from xT) -------------------------
        xs_f = consts.tile([P, KD], F32)
        for c in range(KD):
            nc.vector.tensor_reduce(xs_f[:, c:c + 1], xT_t[:, c, :],
                                    axis=mybir.AxisListType.X, op=ALU.add)
        xsT = consts.tile([P, KD], BF16)
        nc.vector.tensor_copy(xsT[:], xs_f[:])

        # ---- g broadcast to [128, 512] for the xg pass -----------------
        ps_gbc = psum.tile([P, D], F32, tag="bigbank", bufs=2)
        nc.tensor.matmul(ps_gbc[:], ones2[:], g_t[:], start=True, stop=True)
        g_bc = consts.tile([P, D], F32)
        nc.vector.tensor_copy(g_bc[:], ps_gbc[:])

        # ---- weight stream -------------------------------------------
        # Per 512-wide hd block j:
        #   trow_j [1,512] = sum_c xsT[:,c].T @ wv_chunk(c)   (4 fat MMs,
        #     the wv bytes stream through the PE as rhs at 1 col/cycle)
        #   Scalar copies trow_j to SBUF bf16; 4 PE transposes give the
        #   tT columns; one DVE copy lands them in tT.
        #   zrow += tT_col(o).T @ fc_chunk(o)                 (4 fat MMs)
        # z MMs run two blocks behind the t MMs so the PE never waits on
        # the Scalar/DVE copy chain.  zrow accumulates in a single psum
        # bank with one start/stop group.
        trows = consts.tile([1, 4 * NB * P], BF16)     # t row, bf16, S-scaled
        tT = consts.tile([P, 4 * NB], BF16)
        ps_zrow = psum.tile([1, D], F32, tag="zrow", bufs=1)

        def emit_tblock(j):
            wf = wf_tiles[j]
            ps_tr = psum.tile([1, 512], F32, tag="trow", bufs=2)
            for c in range(KD):
                nc.tensor.matmul(ps_tr[:], xsT[:, c:c + 1], wf[:, c, :],
                                 start=(c == 0), stop=(c == KD - 1))
            nc.scalar.activation(trows[0:1, j * 512:(j + 1) * 512], ps_tr[:],
                                 AF.Identity)

        def emit_tpose(j):
            ps_tp = psum.tile([P, 4, 2], BF16, tag="tpose", bufs=1)
            for oc in range(4):
                o = 4 * j + oc
                nc.tensor.transpose(ps_tp[:, oc, 0:1],
                                    trows[0:1, o * P:(o + 1) * P], id1[:])
            nc.scalar.activation(tT[:, 4 * j:4 * j + 4], ps_tp[:, :, 0],
                                 AF.Identity)

        def emit_zblock(j):
            wf = wf_tiles[j]
            for oc in range(4):
                o = 4 * j + oc
                nc.tensor.matmul(ps_zrow[:], tT[:, o:o + 1], wf[:, 4 + oc, :],
                                 start=(o == 0), stop=(o == 4 * NB - 1))

        for j in range(NB):
            emit_tblock(j)
            if j >= 1:
                emit_tpose(j - 1)
            if j >= 2:
                emit_zblock(j - 2)
        emit_tpose(NB - 1)
        emit_zblock(NB - 2)
        emit_zblock(NB - 1)

        # ---- per-token x statistics + xg during the stream -------------
        varx8 = consts.tile([P, NT], F32)
        xg_tiles = []
        for t in range(NT):
            s6 = work.tile([P, 6], F32, tag="s6")
            nc.vector.bn_stats(s6[:], x_t[:, t, :])
            mv = work.tile([P, 2], F32, tag="mv")
            nc.vector.bn_aggr(mv[:], s6[:])
            nc.vector.tensor_copy(varx8[:, t:t + 1], mv[:, 1:2])
            negmx = work.tile([P, 1], F32, tag="negmx")
            nc.vector.tensor_scalar_mul(negmx[:], mv[:, 0:1], -1.0)
            xg = work.tile([P, D], BF16, tag="xg", bufs=8)
            nc.vector.scalar_tensor_tensor(
                xg[:], x_t[:, t, :], negmx[:], g_bc[:],
                op0=ALU.add, op1=ALU.mult)
            xg_tiles.append(xg)

        # b row in bf16 for the tail outer product (early, off critical path)
        brow_bf = consts.tile([1, D], BF16)
        nc.vector.tensor_copy(brow_bf[:], b_t[:])

        # ---- z tail: zrow -> zc row + zc cols, variance pieces ---------
        zrow = consts.tile([1, D], F32)
        zsum = consts.tile([1, 1], F32)
        nc.vector.scalar_tensor_tensor(
            zrow[:], ps_zrow[:], INV_S2, c_t[:], op0=ALU.mult, op1=ALU.add,
            accum_out=zsum[:])
        negmz = consts.tile([1, 1], F32)
        nc.scalar.mul(negmz[:], zsum[:], -1.0 / D)
        zqs = consts.tile([1, 1], F32)
        zsqrow = work.tile([1, D], F32, tag="zsq")
        nc.vector.scalar_tensor_tensor(
            zsqrow[:], zrow[:], 1.0, zrow[:], op0=ALU.mult, op1=ALU.mult,
            accum_out=zqs[:])
        # mean(z^2) - mean(z)^2 path (parallel with the zc path)
        mzsq = consts.tile([1, 1], F32)
        nc.vector.tensor_mul(mzsq[:], negmz[:], negmz[:])
        negmzsq = consts.tile([1, 1], F32)
        nc.vector.tensor_scalar_mul(negmzsq[:], mzsq[:], -1.0)
        ezv = consts.tile([1, 1], F32)
        nc.vector.scalar_tensor_tensor(
            ezv[:], zqs[:], 1.0 / D, negmzsq[:], op0=ALU.mult, op1=ALU.add)
        ps_ez = psum.tile([P, 1], F32, tag="small", bufs=2)
        nc.tensor.matmul(ps_ez[:], ones2[:], ezv[:], start=True, stop=True)
        bias8 = consts.tile([P, 1], F32)
        nc.scalar.activation(bias8[:], ps_ez[:], AF.Identity, bias=eps_t[:],
                             scale=1.0)

        # zc row (bf16) and its columns for the dot products
        zc_row = consts.tile([1, D], BF16)
        nc.scalar.activation(zc_row[:], zrow[:], AF.Identity, bias=negmz[:])
        ps_zc = psum.tile([P, KD, 2], BF16, tag="tpose", bufs=1)
        for r in range(KD):
            nc.tensor.transpose(ps_zc[:, r, 0:1],
                                zc_row[0:1, r * P:(r + 1) * P], id1[:])
        zc4 = consts.tile([P, KD], BF16)
        nc.scalar.activation(zc4[:], ps_zc[:, :, 0], AF.Identity)

        # zg row from zc row
        zgrow = consts.tile([1, D], BF16)
        nc.vector.scalar_tensor_tensor(
            zgrow[:], zc_row[:], 1.0, g_t[:], op0=ALU.mult, op1=ALU.mult)

        # ---- x.zc dots on the PE, batched variance/rstd ----------------
        pd8 = psum.tile([P, NT], F32, tag="trow", bufs=2)
        for t in range(NT):
            for c in range(KD):
                nc.tensor.matmul(
                    pd8[:, t:t + 1],
                    xT_t[:, c, t * P:(t + 1) * P],
                    zc4[:, c:c + 1],
                    start=(c == 0), stop=(c == KD - 1))
        var8 = consts.tile([P, NT], F32)
        nc.vector.scalar_tensor_tensor(
            var8[:], pd8[:], 2.0 / D, varx8[:], op0=ALU.mult, op1=ALU.add)
        std8 = consts.tile([P, NT], F32)
        nc.scalar.activation(std8[:], var8[:], AF.Sqrt, bias=bias8[:])
        rstd8 = consts.tile([P, NT], BF16)
        nc.vector.reciprocal(rstd8[:], std8[:])

        if dbg:
            dxs = consts.tile([P, KD], F32)
            nc.vector.tensor_copy(dxs[:], xsT[:])
            nc.sync.dma_start(dbg_xs.ap(), dxs[:])
            dtT = consts.tile([P, 4 * NB], F32)
            nc.vector.tensor_copy(dtT[:], tT[:])
            nc.sync.dma_start(dbg_tT.ap(), dtT[:])
            dz4 = consts.tile([1, D], F32)
            nc.vector.tensor_copy(dz4[:], zrow[:])
            nc.sync.dma_start(dbg_z4.ap(), dz4[:])
            dr8 = consts.tile([P, NT], F32)
            nc.vector.tensor_copy(dr8[:], rstd8[:])
            nc.sync.dma_start(dbg_r8.ap(), dr8[:])
            dzg = consts.tile([1, D], F32)
            nc.vector.tensor_copy(dzg[:], zgrow[:])
            nc.sync.dma_start(dbg_zg.ap(), dzg[:])

        # rstd rows via one column transpose per tile (base partition 0)
        rstd_rows = []
        for t in range(NT):
            ps_st = psum.tile([1, P], BF16, tag="small", bufs=2)
            nc.tensor.transpose(ps_st[:], rstd8[:, t:t + 1], id_t[:])
            rrow = work.tile([1, P], BF16, tag="rrow", bufs=8)
            nc.scalar.activation(rrow[:], ps_st[:], AF.Identity)
            rstd_rows.append(rrow)

        # ---- final: two outer-product matmuls + one DVE pass per tile --
        obuf = consts.tile([P, NT, D], F16)
        for t in range(NT):
            ps_o = psum.tile([P, D], F32, tag="bigbank", bufs=2)
            nc.tensor.matmul(ps_o[:], ones2b[:], brow_bf[:],
                             start=True, stop=False)
            nc.tensor.matmul(ps_o[:], rstd_rows[t][:], zgrow[:],
                             start=False, stop=True)
            nc.vector.scalar_tensor_tensor(
                obuf[:, t, :], xg_tiles[t][:], rstd8[:, t:t + 1], ps_o[:],
                op0=ALU.mult, op1=ALU.add)
            if t == NT // 2 - 1:
                nc.sync.dma_start(out_v[:, 0:NT // 2, :],
                                  obuf[:, 0:NT // 2, :])
        nc.sync.dma_start(out_v[:, NT // 2:NT, :], obuf[:, NT // 2:NT, :])

    nc.compile()
    return nc


_NC_CACHE = None


def _get_nc():
    global _NC_CACHE
    if _NC_CACHE is None:
        _NC_CACHE = build_kernel()
    return _NC_CACHE


def _shard_inputs(inputs):
    bf = ml_dtypes.bfloat16
    f8 = ml_dtypes.float8_e3m4
    x = np.asarray(inputs["input"], dtype=np.float32)
    wv = np.asarray(inputs["wv"], dtype=np.float32)
    bv = np.asarray(inputs["bv"], dtype=np.float32)
    fc_w = np.asarray(inputs["fc_w"], dtype=np.float32)
    fc_b = np.asarray(inputs["fc_b"], dtype=np.float32)
    ln_g = np.asarray(inputs["ln_g"], dtype=np.float32)
    ln_b = np.asarray(inputs["ln_b"], dtype=np.float32)

    wv_q = (wv * S).astype(f8)
    fc_q = (fc_w * S).astype(f8)
    # wv part:  [j, p, c, m]  = wv_q[c*128 + p, j*512 + m]
    wv_bl = wv_q.reshape(KD, P, NB, 512).transpose(2, 1, 0, 3)
    # fc part:  [j, p, oc, d] = fc_q[(4j + oc)*128 + p, d]
    fc_bl = fc_q.reshape(NB, 4, P, 512).transpose(0, 2, 1, 3)
    wvfc = np.ascontiguousarray(
        np.concatenate([wv_bl, fc_bl], axis=2))               # [8,128,8,512]

    c_vec = (float(L) * bv) @ fc_w + fc_b                     # exact fp32
    crow = np.ascontiguousarray(c_vec[None, :])               # [1, 512]
    grow = np.ascontiguousarray(ln_g[None, :])
    brow = np.ascontiguousarray(ln_b[None, :])
    id128 = np.eye(P, dtype=np.float32).astype(bf)

    in_maps = []
    for i in range(N_CORES):
        xT_bl = np.ascontiguousarray(
            x[i].T.reshape(KD, P, L).transpose(1, 0, 2)).astype(bf)
        x_bl = np.ascontiguousarray(
            x[i].reshape(NT, P, D).transpose(1, 0, 2)).astype(bf)
        in_maps.append({
            "xT": xT_bl,
            "x": x_bl,
            "wvfc": wvfc.reshape(NB, P, 8, 512),
            "id128": id128,
            "grow": grow,
            "brow": brow,
            "crow": crow,
        })
    return in_maps


def kernel(**inputs) -> np.ndarray:
    nc = _get_nc()
    in_maps = _shard_inputs(inputs)
    res = run_bass_kernel_spmd(nc, in_maps, core_ids=list(range(N_CORES)))
    out = np.stack([res.results[i]["out"] for i in range(N_CORES)], axis=0)
    return out.astype(np.float32)


def _install_ntff_hook_shim():
    """Bridge trn_boot's ctypes NTFF profiler into antenv.axon_hooks,
    which bass_utils imports when trace=True under axon."""
    import sys
    import types
    try:
        from antenv.axon_hooks import get_axon_ntff_profile_hook  # noqa: F401
        return
    except ImportError:
        pass
    try:
        from trn_agent_boot.trn_boot import _ntff_profile_via_ctypes
        hook = _ntff_profile_via_ctypes("/opt/axon/libaxon_pjrt.so")
    except Exception:
        hook = None
    mod = types.ModuleType("antenv.axon_hooks")
    state = {"hook": hook}
    mod.get_axon_ntff_profile_hook = lambda: state["hook"]
    mod.set_axon_ntff_profile_hook = lambda h: state.update(hook=h)
    sys.modules["antenv.axon_hooks"] = mod
    import antenv
    antenv.axon_hooks = mod


def kernel_profiled(inputs, trace_cores=None):
    """Like kernel() but with trace=True; returns (out, BassKernelResults)."""
    _install_ntff_hook_shim()
    nc = _get_nc()
    in_maps = _shard_inputs(inputs)
    res = run_bass_kernel_spmd(
        nc, in_maps, core_ids=list(range(N_CORES)), trace=True,
        trace_cores=trace_cores if trace_cores is not None else [0])
    out = np.stack([res.results[i]["out"] for i in range(N_CORES)], axis=0)
    return out.astype(np.float32), res


if __name__ == "__main__":
    import sys
    if "--sim" in sys.argv:
        # quick single-core CoreSim check against the collapsed math
        from concourse.bass_interp import CoreSim
        rng = np.random.default_rng(0)
        x = rng.standard_normal((B, L, D), dtype=np.float32)
        wv = rng.standard_normal((D, HD), dtype=np.float32) * 0.025
        bv = rng.standard_normal(HD, dtype=np.float32) * 0.025
        fc_w = rng.standard_normal((HD, D), dtype=np.float32) * 0.009
        fc_b = rng.standard_normal(D, dtype=np.float32) * 0.015
        g = rng.standard_normal(D, dtype=np.float32) * 0.3 + 1.0
        b = rng.standard_normal(D, dtype=np.float32) * 0.1
        inputs = dict(input=x, wv=wv, bv=bv, fc_w=fc_w, fc_b=fc_b,
                      ln_g=g, ln_b=b)

        nc = _get_nc()
        in_maps = _shard_inputs(inputs)
        sim = CoreSim(nc, trace=False)
        for k, v in in_maps[0].items():
            sim.tensor(k)[:] = v
        sim.simulate()
        got = np.array(sim.tensor("out")).astype(np.float32)

        xsum = x[0].sum(0)
        z = (xsum @ wv + L * bv) @ fc_w + fc_b
        y = x[0] + z[None, :]
        mu = y.mean(-1, keepdims=True)
        var = y.var(-1, keepdims=True)
        want = (y - mu) / np.sqrt(var + EPS) * g + b
        err = np.abs(got - want).max() / np.abs(want).max()
        print("sim absmax rel err:", err)
        assert err < 2e-2, err
        print("SIM PASS")


# revision 32
# speedup vs baseline: 1.9321x; 1.0198x over previous
"""Trainium2 Bass kernel for nn_MultiHeadAttention_26482768347194.

Key algebraic fact: the reference applies softmax over a size-1 trailing
axis, so the attention score matrix is exactly all-ones.  The whole module
collapses (exactly, in real arithmetic) to

    xsum[b]   = sum_l x[b, l, :]                        # (D,)
    t[b]      = xsum[b] @ wv + L * bv                   # (H*D,)
    z[b]      = t[b] @ fc_w + fc_b                      # (D,)
    y[b,l,:]  = x[b,l,:] + z[b]
    out       = LayerNorm(y) * ln_g + ln_b              # over last dim

q/k/tanh/score inputs are mathematically dead.

Sharding: pure data-parallel over batch, one batch element per core,
weights replicated; cross-core collectives cost ~70us under this runtime
(launch-skew barrier) so each core runs fully independently.

v2 design (vs the 64us baseline): the kernel is DMA-stream-bound on the
replicated 8MB bf16 weight load, with a long unoverlapped tail.  Changes:
  * wv / fc_w ship as fp8 e3m4 scaled by 64 (4MB instead of 8MB); PE
    matmuls run fp8-weights x bf16-activations (PE upconverts operands
    independently).  The exact bias path c = (L*bv) @ fc_w + fc_b is
    precomputed in fp32 on the host (it is batch-independent), so only
    the batch-dependent xsum @ wv @ fc term sees quantization.  Measured
    end-to-end absmax rel err of the full rounding model: ~1.1e-2.
  * out ships as fp16 (1MB instead of 2MB fp32), upcast on the host.
  * xsum via DVE free-axis reduces of x.T (frees ~32 PE matmuls).
  * t and z are computed in column form throughout ([128,k] tiles):
    t cols <- wv chunks (lhsT, fp8) x xsumT cols; z cols <- fc chunks
    (lhsT, fp8) x t cols.  No transposes or single-partition row ops on
    the critical path; zc columns feed the x.zc dot products directly.
  * layernorm tail collapsed to ONE DVE pass per token tile:
      out = xg * rstd + PSUM,  PSUM = ones (x) b  +  rstd (x) zg
    built by a single K=2 PE outer-product per tile ([ones; rstd_t]
    stationary, [b; zc*g] moving); xg = (x - mean_x) * ln_g and the
    per-token x statistics are computed on the DVE during the weight
    stream.  var_y = var_x + (2/D) x.zc + mean(z^2) - mean(z)^2.
  * DMA: few fat triggers (xT, x, 8 weight blocks, 2 output halves),
    4KB contiguous per partition per weight block, ordered so the
    weight stream starts immediately behind xT.

This file is self-contained: shapes are hardcoded, no sibling imports.
"""

from contextlib import ExitStack

import numpy as np
import ml_dtypes

import concourse.bass as bass
import concourse.bacc as bacc
import concourse.mybir as mybir
import concourse.tile as tile
from concourse.bass_utils import run_bass_kernel_spmd

B, L, D, H = 8, 1024, 512, 8
HD = H * D          # 4096
P = 128             # partitions
NT = L // P         # 8 token tiles per core
KD = D // P         # 4 contraction chunks over d
NB = HD // 512      # 8 weight blocks (512 hd columns each)
EPS = 1e-5
N_CORES = 8
S = 64.0            # fp8 weight scale
INV_S2 = 1.0 / (S * S)

F32 = mybir.dt.float32
F16 = mybir.dt.float16
BF16 = mybir.dt.bfloat16
F8 = mybir.dt.float8e3
AF = mybir.ActivationFunctionType
ALU = mybir.AluOpType


def build_kernel():
    nc = bacc.Bacc("TRN2", target_bir_lowering=False, debug=False,
                   num_devices=N_CORES)

    # host-blocked layouts; every big DMA reads 4-8KB contiguous per
    # partition row:
    #   xT[p, c, l]     = x[l, c*128 + p]                  (1MB bf16)
    #   x[p, t, d]      = x[t*128 + p, d]                  (1MB bf16)
    #   wvfc[j, p, m]   = 4x512 wv cols + 4x512 fc rows    (4MB fp8)
    #     wv part c*512+m  = wv_q[c*128 + p, j*512 + m]
    #     fc part oc*512+d = fc_q[(4j+oc)*128 + p, d]
    #   gb8 rows 0-3 = ln_g.reshape(4,128), rows 4-7 = ln_b.reshape(4,128)
    #   cT[p, blk]      = c[blk*128 + p],  c = (L*bv) @ fc_w + fc_b
    xT_d = nc.dram_tensor("xT", [P, KD, L], BF16, kind="ExternalInput")
    x_d = nc.dram_tensor("x", [P, NT, D], BF16, kind="ExternalInput")
    wvfc_d = nc.dram_tensor("wvfc", [NB, P, 8, 512], F8, kind="ExternalInput")
    id_d = nc.dram_tensor("id128", [P, P], BF16, kind="ExternalInput")
    g_d = nc.dram_tensor("grow", [1, D], F32, kind="ExternalInput")
    b_d = nc.dram_tensor("brow", [1, D], F32, kind="ExternalInput")
    c_d = nc.dram_tensor("crow", [1, D], F32, kind="ExternalInput")
    out_d = nc.dram_tensor("out", [L, D], F16, kind="ExternalOutput")
    import os
    dbg = os.environ.get("KERNEL_DEBUG_TAPS") == "1"
    if dbg:
        dbg_xs = nc.dram_tensor("dbg_xs", [P, KD], F32, kind="ExternalOutput")
        dbg_tT = nc.dram_tensor("dbg_tT", [P, 4 * NB], F32,
                                kind="ExternalOutput")
        dbg_z4 = nc.dram_tensor("dbg_z4", [1, D], F32, kind="ExternalOutput")
        dbg_r8 = nc.dram_tensor("dbg_r8", [P, NT], F32, kind="ExternalOutput")
        dbg_zg = nc.dram_tensor("dbg_zg", [1, D], F32, kind="ExternalOutput")

    out_v = out_d.ap().rearrange("(t p) d -> p t d", p=P)        # [P, NT, D]

    with tile.TileContext(nc, pool_alloc_mode="queue") as tc, \
            ExitStack() as ctx:
        ctx.enter_context(nc.allow_low_precision(
            reason="bf16 accumulator feeds, validated end-to-end ~1.1e-2"))
        consts = ctx.enter_context(tc.tile_pool(name="consts", bufs=1))
        work = ctx.enter_context(tc.tile_pool(name="work", bufs=3))
        psum = ctx.enter_context(
            tc.tile_pool(name="psum", bufs=1, space=bass.MemorySpace.PSUM))

        # ---- tiny SBUF constants (no DMA) ------------------------------
        ones2 = consts.tile([1, P], F32)         # K=1 broadcast lhsT
        nc.gpsimd.memset(ones2[:], 1.0)
        id1 = consts.tile([1, 1], BF16)          # 1x1 identity (row->col)
        nc.gpsimd.memset(id1[:], 1.0)
        eps_t = consts.tile([P, 1], F32)
        nc.gpsimd.memset(eps_t[:], EPS)
        ones2b = consts.tile([1, P], BF16)       # bf16 K=1 lhsT for b row
        nc.gpsimd.memset(ones2b[:], 1.0)
        # warm the Scalar activation table with Sqrt's function set before
        # any real ACT runs; the mid-tail ACT_TABLE_LOAD (1.3us) disappears
        warm = consts.tile([P, 1], F32)
        nc.scalar.activation(warm[:], eps_t[:], AF.Sqrt)

        # ---- DMA program: xT halves first, weights behind, x halves mid
        xT_t = consts.tile([P, KD, L], BF16)
        nc.sync.dma_start(xT_t[:, 0:2, :], xT_d.ap()[:, 0:2, :])
        nc.sync.dma_start(xT_t[:, 2:4, :], xT_d.ap()[:, 2:4, :])

        wf_tiles = []
        for j in range(NB):
            wf = consts.tile([P, 8, 512], F8, tag="wf", bufs=NB)
            wf_tiles.append(wf)
        nc.sync.dma_start(wf_tiles[0][:], wvfc_d.ap()[0])

        g_t = consts.tile([1, D], F32)
        nc.sync.dma_start(g_t[:], g_d.ap())
        x_t = consts.tile([P, NT, D], BF16)
        nc.sync.dma_start(x_t[:, 0:NT // 2, :], x_d.ap()[:, 0:NT // 2, :])

        nc.sync.dma_start(wf_tiles[1][:], wvfc_d.ap()[1])
        nc.sync.dma_start(wf_tiles[2][:], wvfc_d.ap()[2])
        nc.sync.dma_start(wf_tiles[3][:], wvfc_d.ap()[3])

        nc.sync.dma_start(x_t[:, NT // 2:NT, :], x_d.ap()[:, NT // 2:NT, :])

        for j in range(4, NB):
            nc.sync.dma_start(wf_tiles[j][:], wvfc_d.ap()[j])

        id_t = consts.tile([P, P], BF16)
        nc.sync.dma_start(id_t[:], id_d.ap())
        b_t = consts.tile([1, D], F32)
        nc.sync.dma_start(b_t[:], b_d.ap())
        c_t = consts.tile([1, D], F32)
        nc.sync.dma_start(c_t[:], c_d.ap())

        # ---- xsum columns on the DVE (from xT) -------------------------
        xs_f = consts.tile([P, KD], F32)
        for c in range(KD):
            nc.vector.tensor_reduce(xs_f[:, c:c + 1], xT_t[:, c, :],
                                    axis=mybir.AxisListType.X, op=ALU.add)
        xsT = consts.tile([P, KD], BF16)
        nc.vector.tensor_copy(xsT[:], xs_f[:])

        # ---- g broadcast to [128, 512] for the xg pass -----------------
        ps_gbc = psum.tile([P, D], F32, tag="bigbank", bufs=2)
        nc.tensor.matmul(ps_gbc[:], ones2[:], g_t[:], start=True, stop=True)
        g_bc = consts.tile([P, D], BF16)
        nc.vector.tensor_copy(g_bc[:], ps_gbc[:])

        # ---- weight stream -------------------------------------------
        # Per 512-wide hd block j:
        #   tT cols <- 16 LDW[128x128 fp8]+MM[N=1] pairs (ps_t [128,4]),
        #     ~25-50ns per pair at full pstate; Scalar lands them in tT.
        #   zrow    <- 4 fat MMs: lhsT = tT col, rhs = fc chunk [128,512]
        #     accumulated across all 32 chunks in one psum bank.
        # z MMs run two blocks behind so the PE never waits on the Scalar
        # copy.
        tT = consts.tile([P, 4 * NB], BF16)
        ps_zrow = psum.tile([1, D], F32, tag="zrow", bufs=1)

        def emit_tblock(j):
            wf = wf_tiles[j]
            ps_t = psum.tile([P, 4], F32, tag="tcols", bufs=2)
            for oc in range(4):
                for c in range(KD):
                    nc.tensor.matmul(
                        ps_t[:, oc:oc + 1],
                        wf[:, c, oc * P:(oc + 1) * P],
                        xsT[:, c:c + 1],
                        start=(c == 0), stop=(c == KD - 1))
            nc.scalar.activation(tT[:, 4 * j:4 * j + 4], ps_t[:],
                                 AF.Identity)

        def emit_zblock(j):
            wf = wf_tiles[j]
            for oc in range(4):
                o = 4 * j + oc
                nc.tensor.matmul(ps_zrow[:], tT[:, o:o + 1], wf[:, 4 + oc, :],
                                 start=(o == 0), stop=(o == 4 * NB - 1))

        for j in range(NB):
            emit_tblock(j)
            if j >= 2:
                emit_zblock(j - 2)
        emit_zblock(NB - 2)
        emit_zblock(NB - 1)

        # ---- per-token x statistics + xg during the stream -------------
        varx8 = consts.tile([P, NT], F32)
        xg_tiles = []
        for t in range(NT):
            s6 = work.tile([P, 6], F32, tag="s6")
            nc.vector.bn_stats(s6[:], x_t[:, t, :])
            mv = work.tile([P, 2], F32, tag="mv")
            nc.vector.bn_aggr(mv[:], s6[:])
            nc.vector.tensor_copy(varx8[:, t:t + 1], mv[:, 1:2])
            negmx = work.tile([P, 1], F32, tag="negmx")
            nc.vector.tensor_scalar_mul(negmx[:], mv[:, 0:1], -1.0)
            xg = work.tile([P, D], BF16, tag="xg", bufs=8)
            nc.vector.scalar_tensor_tensor(
                xg[:], x_t[:, t, :], negmx[:], g_bc[:],
                op0=ALU.add, op1=ALU.mult)
            xg_tiles.append(xg)

        # b row in bf16 for the tail outer product (early, off critical path)
        brow_bf = consts.tile([1, D], BF16)
        nc.vector.tensor_copy(brow_bf[:], b_t[:])

        # ---- z tail: zrow -> zc row + zc cols, variance pieces ---------
        zrow = consts.tile([1, D], F32)
        zsum = consts.tile([1, 1], F32)
        nc.vector.scalar_tensor_tensor(
            zrow[:], ps_zrow[:], INV_S2, c_t[:], op0=ALU.mult, op1=ALU.add,
            accum_out=zsum[:])
        negmz = consts.tile([1, 1], F32)
        nc.scalar.mul(negmz[:], zsum[:], -1.0 / D)
        zqs = consts.tile([1, 1], F32)
        zsqrow = work.tile([1, D], F32, tag="zsq")
        nc.vector.scalar_tensor_tensor(
            zsqrow[:], zrow[:], 1.0, zrow[:], op0=ALU.mult, op1=ALU.mult,
            accum_out=zqs[:])
        # mean(z^2) - mean(z)^2 path (parallel with the zc path)
        mzsq = consts.tile([1, 1], F32)
        nc.vector.tensor_mul(mzsq[:], negmz[:], negmz[:])
        negmzsq = consts.tile([1, 1], F32)
        nc.vector.tensor_scalar_mul(negmzsq[:], mzsq[:], -1.0)
        ezv = consts.tile([1, 1], F32)
        nc.vector.scalar_tensor_tensor(
            ezv[:], zqs[:], 1.0 / D, negmzsq[:], op0=ALU.mult, op1=ALU.add)
        ps_ez = psum.tile([P, 1], F32, tag="small", bufs=2)
        nc.tensor.matmul(ps_ez[:], ones2[:], ezv[:], start=True, stop=True)
        bias8 = consts.tile([P, 1], F32)
        nc.scalar.activation(bias8[:], ps_ez[:], AF.Identity, bias=eps_t[:],
                             scale=1.0)

        # zc row (bf16) and its columns for the dot products
        zc_row = consts.tile([1, D], BF16)
        nc.scalar.activation(zc_row[:], zrow[:], AF.Identity, bias=negmz[:])
        ps_zc = psum.tile([P, KD, 2], BF16, tag="tcols", bufs=2)
        for r in range(KD):
            nc.tensor.transpose(ps_zc[:, r, 0:1],
                                zc_row[0:1, r * P:(r + 1) * P], id1[:])
        zc4 = consts.tile([P, KD], BF16)
        nc.scalar.activation(zc4[:], ps_zc[:, :, 0], AF.Identity)

        # zg row from zc row
        zgrow = consts.tile([1, D], BF16)
        nc.vector.scalar_tensor_tensor(
            zgrow[:], zc_row[:], 1.0, g_t[:], op0=ALU.mult, op1=ALU.mult)

        # ---- x.zc dots on the PE, batched variance/rstd ----------------
        pd8 = psum.tile([P, NT], F32, tag="pd", bufs=1)
        for t in range(NT):
            for c in range(KD):
                nc.tensor.matmul(
                    pd8[:, t:t + 1],
                    xT_t[:, c, t * P:(t + 1) * P],
                    zc4[:, c:c + 1],
                    start=(c == 0), stop=(c == KD - 1))
        var8 = consts.tile([P, NT], F32)
        nc.vector.scalar_tensor_tensor(
            var8[:], pd8[:], 2.0 / D, varx8[:], op0=ALU.mult, op1=ALU.add)
        std8 = consts.tile([P, NT], F32)
        nc.scalar.activation(std8[:], var8[:], AF.Sqrt, bias=bias8[:])
        rstd8 = consts.tile([P, NT], BF16)
        nc.vector.reciprocal(rstd8[:], std8[:])

        if dbg:
            dxs = consts.tile([P, KD], F32)
            nc.vector.tensor_copy(dxs[:], xsT[:])
            nc.sync.dma_start(dbg_xs.ap(), dxs[:])
            dtT = consts.tile([P, 4 * NB], F32)
            nc.vector.tensor_copy(dtT[:], tT[:])
            nc.sync.dma_start(dbg_tT.ap(), dtT[:])
            dz4 = consts.tile([1, D], F32)
            nc.vector.tensor_copy(dz4[:], zrow[:])
            nc.sync.dma_start(dbg_z4.ap(), dz4[:])
            dr8 = consts.tile([P, NT], F32)
            nc.vector.tensor_copy(dr8[:], rstd8[:])
            nc.sync.dma_start(dbg_r8.ap(), dr8[:])
            dzg = consts.tile([1, D], F32)
            nc.vector.tensor_copy(dzg[:], zgrow[:])
            nc.sync.dma_start(dbg_zg.ap(), dzg[:])

        # rstd rows via one column transpose per tile (base partition 0)
        rstd_rows = []
        for t in range(NT):
            ps_st = psum.tile([1, P], BF16, tag="small", bufs=2)
            nc.tensor.transpose(ps_st[:], rstd8[:, t:t + 1], id_t[:])
            rrow = work.tile([1, P], BF16, tag="rrow", bufs=8)
            nc.scalar.activation(rrow[:], ps_st[:], AF.Identity)
            rstd_rows.append(rrow)

        # ---- final: two outer-product matmuls + one elementwise pass ---
        # per tile; finals alternate DVE / GpSimd so the two engines each
        # carry half the serial chain
        obuf = consts.tile([P, NT, D], F16)
        for t in range(NT):
            ps_o = psum.tile([P, D], F32, tag="bigbank", bufs=2)
            nc.tensor.matmul(ps_o[:], ones2b[:], brow_bf[:],
                             start=True, stop=False)
            nc.tensor.matmul(ps_o[:], rstd_rows[t][:], zgrow[:],
                             start=False, stop=True)
            eng = nc.vector
            eng.scalar_tensor_tensor(
                obuf[:, t, :], xg_tiles[t][:], rstd8[:, t:t + 1], ps_o[:],
                op0=ALU.mult, op1=ALU.add)
            if t % 2 == 1:
                nc.sync.dma_start(out_v[:, t - 1:t + 1, :],
                                  obuf[:, t - 1:t + 1, :])

    nc.compile()
    return nc


_NC_CACHE = None


def _get_nc():
    global _NC_CACHE
    if _NC_CACHE is None:
        _NC_CACHE = build_kernel()
    return _NC_CACHE


def _shard_inputs(inputs):
    bf = ml_dtypes.bfloat16
    f8 = ml_dtypes.float8_e3m4
    x = np.asarray(inputs["input"], dtype=np.float32)
    wv = np.asarray(inputs["wv"], dtype=np.float32)
    bv = np.asarray(inputs["bv"], dtype=np.float32)
    fc_w = np.asarray(inputs["fc_w"], dtype=np.float32)
    fc_b = np.asarray(inputs["fc_b"], dtype=np.float32)
    ln_g = np.asarray(inputs["ln_g"], dtype=np.float32)
    ln_b = np.asarray(inputs["ln_b"], dtype=np.float32)

    wv_q = (wv * S).astype(f8)
    fc_q = (fc_w * S).astype(f8)
    # wv part:  [j, p, c, m]  = wv_q[c*128 + p, j*512 + m]
    wv_bl = wv_q.reshape(KD, P, NB, 512).transpose(2, 1, 0, 3)
    # fc part:  [j, p, oc, d] = fc_q[(4j + oc)*128 + p, d]
    fc_bl = fc_q.reshape(NB, 4, P, 512).transpose(0, 2, 1, 3)
    wvfc = np.ascontiguousarray(
        np.concatenate([wv_bl, fc_bl], axis=2))               # [8,128,8,512]

    c_vec = (float(L) * bv) @ fc_w + fc_b                     # exact fp32
    crow = np.ascontiguousarray(c_vec[None, :])               # [1, 512]
    grow = np.ascontiguousarray(ln_g[None, :])
    brow = np.ascontiguousarray(ln_b[None, :])
    id128 = np.eye(P, dtype=np.float32).astype(bf)

    in_maps = []
    for i in range(N_CORES):
        xT_bl = np.ascontiguousarray(
            x[i].T.reshape(KD, P, L).transpose(1, 0, 2)).astype(bf)
        x_bl = np.ascontiguousarray(
            x[i].reshape(NT, P, D).transpose(1, 0, 2)).astype(bf)
        in_maps.append({
            "xT": xT_bl,
            "x": x_bl,
            "wvfc": wvfc.reshape(NB, P, 8, 512),
            "id128": id128,
            "grow": grow,
            "brow": brow,
            "crow": crow,
        })
    return in_maps


def kernel(**inputs) -> np.ndarray:
    nc = _get_nc()
    in_maps = _shard_inputs(inputs)
    res = run_bass_kernel_spmd(nc, in_maps, core_ids=list(range(N_CORES)))
    out = np.stack([res.results[i]["out"] for i in range(N_CORES)], axis=0)
    return out.astype(np.float32)


def _install_ntff_hook_shim():
    """Bridge trn_boot's ctypes NTFF profiler into antenv.axon_hooks,
    which bass_utils imports when trace=True under axon."""
    import sys
    import types
    try:
        from antenv.axon_hooks import get_axon_ntff_profile_hook  # noqa: F401
        return
    except ImportError:
        pass
    try:
        from trn_agent_boot.trn_boot import _ntff_profile_via_ctypes
        hook = _ntff_profile_via_ctypes("/opt/axon/libaxon_pjrt.so")
    except Exception:
        hook = None
    mod = types.ModuleType("antenv.axon_hooks")
    state = {"hook": hook}
    mod.get_axon_ntff_profile_hook = lambda: state["hook"]
    mod.set_axon_ntff_profile_hook = lambda h: state.update(hook=h)
    sys.modules["antenv.axon_hooks"] = mod
    import antenv
    antenv.axon_hooks = mod


def kernel_profiled(inputs, trace_cores=None):
    """Like kernel() but with trace=True; returns (out, BassKernelResults)."""
    _install_ntff_hook_shim()
    nc = _get_nc()
    in_maps = _shard_inputs(inputs)
    res = run_bass_kernel_spmd(
        nc, in_maps, core_ids=list(range(N_CORES)), trace=True,
        trace_cores=trace_cores if trace_cores is not None else [0])
    out = np.stack([res.results[i]["out"] for i in range(N_CORES)], axis=0)
    return out.astype(np.float32), res


if __name__ == "__main__":
    import sys
    if "--sim" in sys.argv:
        # quick single-core CoreSim check against the collapsed math
        from concourse.bass_interp import CoreSim
        rng = np.random.default_rng(0)
        x = rng.standard_normal((B, L, D), dtype=np.float32)
        wv = rng.standard_normal((D, HD), dtype=np.float32) * 0.025
        bv = rng.standard_normal(HD, dtype=np.float32) * 0.025
        fc_w = rng.standard_normal((HD, D), dtype=np.float32) * 0.009
        fc_b = rng.standard_normal(D, dtype=np.float32) * 0.015
        g = rng.standard_normal(D, dtype=np.float32) * 0.3 + 1.0
        b = rng.standard_normal(D, dtype=np.float32) * 0.1
        inputs = dict(input=x, wv=wv, bv=bv, fc_w=fc_w, fc_b=fc_b,
                      ln_g=g, ln_b=b)

        nc = _get_nc()
        in_maps = _shard_inputs(inputs)
        sim = CoreSim(nc, trace=False)
        for k, v in in_maps[0].items():
            sim.tensor(k)[:] = v
        sim.simulate()
        got = np.array(sim.tensor("out")).astype(np.float32)

        xsum = x[0].sum(0)
        z = (xsum @ wv + L * bv) @ fc_w + fc_b
        y = x[0] + z[None, :]
        mu = y.mean(-1, keepdims=True)
        var = y.var(-1, keepdims=True)
        want = (y - mu) / np.sqrt(var + EPS) * g + b
        err = np.abs(got - want).max() / np.abs(want).max()
        print("sim absmax rel err:", err)
        assert err < 2e-2, err
        print("SIM PASS")


# revision 33
# speedup vs baseline: 2.0643x; 1.0684x over previous
"""Trainium2 Bass kernel for nn_MultiHeadAttention_26482768347194.

Key algebraic fact: the reference applies softmax over a size-1 trailing
axis, so the attention score matrix is exactly all-ones.  The whole module
collapses (exactly, in real arithmetic) to

    xsum[b]   = sum_l x[b, l, :]                        # (D,)
    t[b]      = xsum[b] @ wv + L * bv                   # (H*D,)
    z[b]      = t[b] @ fc_w + fc_b                      # (D,)
    y[b,l,:]  = x[b,l,:] + z[b]
    out       = LayerNorm(y) * ln_g + ln_b              # over last dim

q/k/tanh/score inputs are mathematically dead.

Sharding: pure data-parallel over batch, one batch element per core,
weights replicated; cross-core collectives cost ~70us under this runtime
(launch-skew barrier) so each core runs fully independently.

v2 design (vs the 64us baseline): the kernel is DMA-stream-bound on the
replicated 8MB bf16 weight load, with a long unoverlapped tail.  Changes:
  * wv / fc_w ship as fp8 e3m4 scaled by 64 (4MB instead of 8MB); PE
    matmuls run fp8-weights x bf16-activations (PE upconverts operands
    independently).  The exact bias path c = (L*bv) @ fc_w + fc_b is
    precomputed in fp32 on the host (it is batch-independent), so only
    the batch-dependent xsum @ wv @ fc term sees quantization.  Measured
    end-to-end absmax rel err of the full rounding model: ~1.1e-2.
  * out ships as fp16 (1MB instead of 2MB fp32), upcast on the host.
  * xsum via DVE free-axis reduces of x.T (frees ~32 PE matmuls).
  * t and z are computed in column form throughout ([128,k] tiles):
    t cols <- wv chunks (lhsT, fp8) x xsumT cols; z cols <- fc chunks
    (lhsT, fp8) x t cols.  No transposes or single-partition row ops on
    the critical path; zc columns feed the x.zc dot products directly.
  * layernorm tail collapsed to ONE DVE pass per token tile:
      out = xg * rstd + PSUM,  PSUM = ones (x) b  +  rstd (x) zg
    built by a single K=2 PE outer-product per tile ([ones; rstd_t]
    stationary, [b; zc*g] moving); xg = (x - mean_x) * ln_g and the
    per-token x statistics are computed on the DVE during the weight
    stream.  var_y = var_x + (2/D) x.zc + mean(z^2) - mean(z)^2.
  * DMA: few fat triggers (xT, x, 8 weight blocks, 2 output halves),
    4KB contiguous per partition per weight block, ordered so the
    weight stream starts immediately behind xT.

This file is self-contained: shapes are hardcoded, no sibling imports.
"""

from contextlib import ExitStack

import numpy as np
import ml_dtypes

import concourse.bass as bass
import concourse.bacc as bacc
import concourse.mybir as mybir
import concourse.tile as tile
from concourse.bass_utils import run_bass_kernel_spmd

B, L, D, H = 8, 1024, 512, 8
HD = H * D          # 4096
P = 128             # partitions
NT = L // P         # 8 token tiles per core
KD = D // P         # 4 contraction chunks over d
NB = HD // 512      # 8 weight blocks (512 hd columns each)
EPS = 1e-5
N_CORES = 8
S = 64.0            # fp8 weight scale
INV_S2 = 1.0 / (S * S)

F32 = mybir.dt.float32
F16 = mybir.dt.float16
BF16 = mybir.dt.bfloat16
F8 = mybir.dt.float8e3
AF = mybir.ActivationFunctionType
ALU = mybir.AluOpType


def build_kernel():
    nc = bacc.Bacc("TRN2", target_bir_lowering=False, debug=False,
                   num_devices=N_CORES)

    # host-blocked layouts; every big DMA reads 4-8KB contiguous per
    # partition row:
    #   xT[p, c, l]     = x[l, c*128 + p]                  (1MB bf16)
    #   x[p, t, d]      = x[t*128 + p, d]                  (1MB bf16)
    #   wvfc[j, p, m]   = 4x512 wv cols + 4x512 fc rows    (4MB fp8)
    #     wv part c*512+m  = wv_q[c*128 + p, j*512 + m]
    #     fc part oc*512+d = fc_q[(4j+oc)*128 + p, d]
    #   gb8 rows 0-3 = ln_g.reshape(4,128), rows 4-7 = ln_b.reshape(4,128)
    #   cT[p, blk]      = c[blk*128 + p],  c = (L*bv) @ fc_w + fc_b
    xT_d = nc.dram_tensor("xT", [P, KD, L], BF16, kind="ExternalInput")
    x_d = nc.dram_tensor("x", [P, NT, D], BF16, kind="ExternalInput")
    wvfc_d = nc.dram_tensor("wvfc", [NB, P, 8, 512], F8, kind="ExternalInput")
    id_d = nc.dram_tensor("id128", [P, P], BF16, kind="ExternalInput")
    g_d = nc.dram_tensor("grow", [1, D], F32, kind="ExternalInput")
    b_d = nc.dram_tensor("brow", [1, D], F32, kind="ExternalInput")
    c_d = nc.dram_tensor("crow", [1, D], F32, kind="ExternalInput")
    out_d = nc.dram_tensor("out", [L, D], F16, kind="ExternalOutput")
    import os
    dbg = os.environ.get("KERNEL_DEBUG_TAPS") == "1"
    if dbg:
        dbg_xs = nc.dram_tensor("dbg_xs", [P, KD], F32, kind="ExternalOutput")
        dbg_tT = nc.dram_tensor("dbg_tT", [P, 4 * NB], F32,
                                kind="ExternalOutput")
        dbg_z4 = nc.dram_tensor("dbg_z4", [1, D], F32, kind="ExternalOutput")
        dbg_r8 = nc.dram_tensor("dbg_r8", [P, NT], F32, kind="ExternalOutput")
        dbg_zg = nc.dram_tensor("dbg_zg", [1, D], F32, kind="ExternalOutput")

    out_v = out_d.ap().rearrange("(t p) d -> p t d", p=P)        # [P, NT, D]

    with tile.TileContext(nc, pool_alloc_mode="queue") as tc, \
            ExitStack() as ctx:
        ctx.enter_context(nc.allow_low_precision(
            reason="bf16 accumulator feeds, validated end-to-end ~1.1e-2"))
        consts = ctx.enter_context(tc.tile_pool(name="consts", bufs=1))
        work = ctx.enter_context(tc.tile_pool(name="work", bufs=3))
        psum = ctx.enter_context(
            tc.tile_pool(name="psum", bufs=1, space=bass.MemorySpace.PSUM))

        # ---- tiny SBUF constants (no DMA) ------------------------------
        ones2 = consts.tile([1, P], F32)         # K=1 broadcast lhsT
        nc.gpsimd.memset(ones2[:], 1.0)
        id1 = consts.tile([1, 1], BF16)          # 1x1 identity (row->col)
        nc.gpsimd.memset(id1[:], 1.0)
        id1f = consts.tile([1, 1], F32)          # f32 variant for f32 rows
        nc.gpsimd.memset(id1f[:], 1.0)
        eps_t = consts.tile([P, 1], F32)
        nc.gpsimd.memset(eps_t[:], EPS)
        ones2b = consts.tile([1, P], BF16)       # bf16 K=1 lhsT for b row
        nc.gpsimd.memset(ones2b[:], 1.0)
        # warm the Scalar activation table with Sqrt's function set before
        # any real ACT runs; the mid-tail ACT_TABLE_LOAD (1.3us) disappears
        warm = consts.tile([P, 1], F32)
        nc.scalar.activation(warm[:], eps_t[:], AF.Sqrt)

        # ---- DMA program: xT halves first, weights behind, x halves mid
        xT_t = consts.tile([P, KD, L], BF16)
        nc.sync.dma_start(xT_t[:, 0:2, :], xT_d.ap()[:, 0:2, :])
        nc.sync.dma_start(xT_t[:, 2:4, :], xT_d.ap()[:, 2:4, :])

        wf_tiles = []
        for j in range(NB):
            wf = consts.tile([P, 8, 512], F8, tag="wf", bufs=NB)
            wf_tiles.append(wf)
        nc.sync.dma_start(wf_tiles[0][:], wvfc_d.ap()[0])

        g_t = consts.tile([1, D], F32)
        nc.sync.dma_start(g_t[:], g_d.ap())
        x_t = consts.tile([P, NT, D], BF16)
        nc.sync.dma_start(x_t[:, 0:NT // 2, :], x_d.ap()[:, 0:NT // 2, :])

        nc.sync.dma_start(wf_tiles[1][:], wvfc_d.ap()[1])
        nc.sync.dma_start(wf_tiles[2][:], wvfc_d.ap()[2])
        nc.sync.dma_start(wf_tiles[3][:], wvfc_d.ap()[3])

        nc.sync.dma_start(x_t[:, NT // 2:NT, :], x_d.ap()[:, NT // 2:NT, :])

        for j in range(4, NB):
            nc.sync.dma_start(wf_tiles[j][:], wvfc_d.ap()[j])

        id_t = consts.tile([P, P], BF16)
        nc.sync.dma_start(id_t[:], id_d.ap())
        b_t = consts.tile([1, D], F32)
        nc.sync.dma_start(b_t[:], b_d.ap())
        c_t = consts.tile([1, D], F32)
        nc.sync.dma_start(c_t[:], c_d.ap())

        # ---- xsum columns on the DVE (from xT) -------------------------
        xs_f = consts.tile([P, KD], F32)
        for c in range(KD):
            nc.vector.tensor_reduce(xs_f[:, c:c + 1], xT_t[:, c, :],
                                    axis=mybir.AxisListType.X, op=ALU.add)
        xsT = consts.tile([P, KD], BF16)
        nc.vector.tensor_copy(xsT[:], xs_f[:])

        # ---- g broadcast to [128, 512] for the xg pass -----------------
        ps_gbc = psum.tile([P, D], F32, tag="bigbank", bufs=2)
        nc.tensor.matmul(ps_gbc[:], ones2[:], g_t[:], start=True, stop=True)
        g_bc = consts.tile([P, D], BF16)
        nc.vector.tensor_copy(g_bc[:], ps_gbc[:])

        # ---- weight stream -------------------------------------------
        # Per 512-wide hd block j:
        #   tT cols <- 16 LDW[128x128 fp8]+MM[N=1] pairs (ps_t [128,4]),
        #     ~25-50ns per pair at full pstate; Scalar lands them in tT.
        #   zrow    <- 4 fat MMs: lhsT = tT col, rhs = fc chunk [128,512]
        #     accumulated across all 32 chunks in one psum bank.
        # z MMs run two blocks behind so the PE never waits on the Scalar
        # copy.
        ps_zrow = psum.tile([1, D], F32, tag="zrow", bufs=1)
        tT_tiles = []

        def emit_tblock(j):
            wf = wf_tiles[j]
            ps_t = psum.tile([P, 4], F32, tag="tcols", bufs=2)
            for oc in range(4):
                for c in range(KD):
                    nc.tensor.matmul(
                        ps_t[:, oc:oc + 1],
                        wf[:, c, oc * P:(oc + 1) * P],
                        xsT[:, c:c + 1],
                        start=(c == 0), stop=(c == KD - 1))
            tTb = work.tile([P, 4], BF16, tag="tTb", bufs=NB)
            nc.scalar.activation(tTb[:], ps_t[:], AF.Identity)
            tT_tiles.append(tTb)

        def emit_zblock(j):
            wf = wf_tiles[j]
            for oc in range(4):
                o = 4 * j + oc
                nc.tensor.matmul(ps_zrow[:], tT_tiles[j][:, oc:oc + 1],
                                 wf[:, 4 + oc, :],
                                 start=(o == 0), stop=(o == 4 * NB - 1))

        for j in range(NB):
            emit_tblock(j)
            if j >= 2:
                emit_zblock(j - 2)
        emit_zblock(NB - 2)
        emit_zblock(NB - 1)

        # ---- per-token x statistics + xg during the stream -------------
        varx8 = consts.tile([P, NT], F32)
        xg_tiles = []
        for t in range(NT):
            s6 = work.tile([P, 6], F32, tag="s6")
            nc.vector.bn_stats(s6[:], x_t[:, t, :])
            mv = work.tile([P, 2], F32, tag="mv")
            nc.vector.bn_aggr(mv[:], s6[:])
            nc.vector.tensor_copy(varx8[:, t:t + 1], mv[:, 1:2])
            negmx = work.tile([P, 1], F32, tag="negmx")
            nc.vector.tensor_scalar_mul(negmx[:], mv[:, 0:1], -1.0)
            xg = work.tile([P, D], BF16, tag="xg", bufs=8)
            nc.vector.scalar_tensor_tensor(
                xg[:], x_t[:, t, :], negmx[:], g_bc[:],
                op0=ALU.add, op1=ALU.mult)
            xg_tiles.append(xg)

        # b row in bf16 for the tail outer product (early, off critical path)
        brow_bf = consts.tile([1, D], BF16)
        nc.vector.tensor_copy(brow_bf[:], b_t[:])

        # ---- z tail: zrow -> zc row + zc cols, variance pieces ---------
        zrow = consts.tile([1, D], F32)
        zsum = consts.tile([1, 1], F32)
        nc.vector.scalar_tensor_tensor(
            zrow[:], ps_zrow[:], INV_S2, c_t[:], op0=ALU.mult, op1=ALU.add,
            accum_out=zsum[:])
        negmz = consts.tile([1, 1], F32)
        nc.scalar.mul(negmz[:], zsum[:], -1.0 / D)
        zqs = consts.tile([1, 1], F32)
        zsqrow = work.tile([1, D], F32, tag="zsq")
        nc.vector.scalar_tensor_tensor(
            zsqrow[:], zrow[:], 1.0, zrow[:], op0=ALU.mult, op1=ALU.mult,
            accum_out=zqs[:])
        # mean(z^2) - mean(z)^2 path (parallel with the zc path)
        mzsq = consts.tile([1, 1], F32)
        nc.vector.tensor_mul(mzsq[:], negmz[:], negmz[:])
        negmzsq = consts.tile([1, 1], F32)
        nc.vector.tensor_scalar_mul(negmzsq[:], mzsq[:], -1.0)
        ezv = consts.tile([1, 1], F32)
        nc.vector.scalar_tensor_tensor(
            ezv[:], zqs[:], 1.0 / D, negmzsq[:], op0=ALU.mult, op1=ALU.add)
        ps_ez = psum.tile([P, 1], F32, tag="small", bufs=2)
        nc.tensor.matmul(ps_ez[:], ones2[:], ezv[:], start=True, stop=True)
        bias8 = consts.tile([P, 1], F32)
        nc.scalar.activation(bias8[:], ps_ez[:], AF.Identity, bias=eps_t[:],
                             scale=1.0)

        # zc columns: transpose zrow directly (f32), apply -mz as a column
        # bias; the zc row never has to exist
        ps_zc = psum.tile([P, KD], F32, tag="tcols", bufs=2)
        for r in range(KD):
            nc.tensor.transpose(ps_zc[:, r:r + 1],
                                zrow[0:1, r * P:(r + 1) * P], id1f[:])
        ps_mz = psum.tile([P, 1], F32, tag="small", bufs=2)
        nc.tensor.matmul(ps_mz[:], ones2[:], negmz[:], start=True, stop=True)
        negmz_bc = consts.tile([P, 1], F32)
        nc.vector.tensor_copy(negmz_bc[:], ps_mz[:])
        zc4 = consts.tile([P, KD], BF16)
        nc.scalar.activation(zc4[:], ps_zc[:], AF.Identity, bias=negmz_bc[:])

        # zg row straight from zrow: (zrow - mz) * g in one pass
        zgrow = consts.tile([1, D], BF16)
        nc.vector.scalar_tensor_tensor(
            zgrow[:], zrow[:], negmz[:], g_t[:], op0=ALU.add, op1=ALU.mult)

        # ---- x.zc dots on the PE, batched variance/rstd ----------------
        pd8 = psum.tile([P, NT], F32, tag="pd", bufs=1)
        for t in range(NT):
            for c in range(KD):
                nc.tensor.matmul(
                    pd8[:, t:t + 1],
                    xT_t[:, c, t * P:(t + 1) * P],
                    zc4[:, c:c + 1],
                    start=(c == 0), stop=(c == KD - 1))
        var8 = consts.tile([P, NT], F32)
        nc.vector.scalar_tensor_tensor(
            var8[:], pd8[:], 2.0 / D, varx8[:], op0=ALU.mult, op1=ALU.add)
        std8 = consts.tile([P, NT], F32)
        nc.scalar.activation(std8[:], var8[:], AF.Sqrt, bias=bias8[:])
        rstd8 = consts.tile([P, NT], BF16)
        nc.vector.reciprocal(rstd8[:], std8[:])

        if dbg:
            dxs = consts.tile([P, KD], F32)
            nc.vector.tensor_copy(dxs[:], xsT[:])
            nc.sync.dma_start(dbg_xs.ap(), dxs[:])
            dtT = consts.tile([P, 4 * NB], F32)
            nc.vector.tensor_copy(dtT[:], tT[:])
            nc.sync.dma_start(dbg_tT.ap(), dtT[:])
            dz4 = consts.tile([1, D], F32)
            nc.vector.tensor_copy(dz4[:], zrow[:])
            nc.sync.dma_start(dbg_z4.ap(), dz4[:])
            dr8 = consts.tile([P, NT], F32)
            nc.vector.tensor_copy(dr8[:], rstd8[:])
            nc.sync.dma_start(dbg_r8.ap(), dr8[:])
            dzg = consts.tile([1, D], F32)
            nc.vector.tensor_copy(dzg[:], zgrow[:])
            nc.sync.dma_start(dbg_zg.ap(), dzg[:])

        # rstd rows via one column transpose per tile (base partition 0)
        rstd_rows = []
        for t in range(NT):
            ps_st = psum.tile([1, P], BF16, tag="small", bufs=2)
            nc.tensor.transpose(ps_st[:], rstd8[:, t:t + 1], id_t[:])
            rrow = work.tile([1, P], BF16, tag="rrow", bufs=8)
            nc.scalar.activation(rrow[:], ps_st[:], AF.Identity)
            rstd_rows.append(rrow)

        # ---- final: two outer-product matmuls + one elementwise pass ---
        # per tile; finals alternate DVE / GpSimd so the two engines each
        # carry half the serial chain
        obuf = consts.tile([P, NT, D], F16)
        for t in range(NT):
            ps_o = psum.tile([P, D], F32, tag="bigbank", bufs=2)
            nc.tensor.matmul(ps_o[:], ones2b[:], brow_bf[:],
                             start=True, stop=False)
            nc.tensor.matmul(ps_o[:], rstd_rows[t][:], zgrow[:],
                             start=False, stop=True)
            eng = nc.vector
            eng.scalar_tensor_tensor(
                obuf[:, t, :], xg_tiles[t][:], rstd8[:, t:t + 1], ps_o[:],
                op0=ALU.mult, op1=ALU.add)
            if t % 2 == 1:
                nc.sync.dma_start(out_v[:, t - 1:t + 1, :],
                                  obuf[:, t - 1:t + 1, :])

    nc.compile()
    return nc


_NC_CACHE = None


def _get_nc():
    global _NC_CACHE
    if _NC_CACHE is None:
        _NC_CACHE = build_kernel()
    return _NC_CACHE


def _shard_inputs(inputs):
    bf = ml_dtypes.bfloat16
    f8 = ml_dtypes.float8_e3m4
    x = np.asarray(inputs["input"], dtype=np.float32)
    wv = np.asarray(inputs["wv"], dtype=np.float32)
    bv = np.asarray(inputs["bv"], dtype=np.float32)
    fc_w = np.asarray(inputs["fc_w"], dtype=np.float32)
    fc_b = np.asarray(inputs["fc_b"], dtype=np.float32)
    ln_g = np.asarray(inputs["ln_g"], dtype=np.float32)
    ln_b = np.asarray(inputs["ln_b"], dtype=np.float32)

    wv_q = (wv * S).astype(f8)
    fc_q = (fc_w * S).astype(f8)
    # wv part:  [j, p, c, m]  = wv_q[c*128 + p, j*512 + m]
    wv_bl = wv_q.reshape(KD, P, NB, 512).transpose(2, 1, 0, 3)
    # fc part:  [j, p, oc, d] = fc_q[(4j + oc)*128 + p, d]
    fc_bl = fc_q.reshape(NB, 4, P, 512).transpose(0, 2, 1, 3)
    wvfc = np.ascontiguousarray(
        np.concatenate([wv_bl, fc_bl], axis=2))               # [8,128,8,512]

    c_vec = (float(L) * bv) @ fc_w + fc_b                     # exact fp32
    crow = np.ascontiguousarray(c_vec[None, :])               # [1, 512]
    grow = np.ascontiguousarray(ln_g[None, :])
    brow = np.ascontiguousarray(ln_b[None, :])
    id128 = np.eye(P, dtype=np.float32).astype(bf)

    in_maps = []
    for i in range(N_CORES):
        xT_bl = np.ascontiguousarray(
            x[i].T.reshape(KD, P, L).transpose(1, 0, 2)).astype(bf)
        x_bl = np.ascontiguousarray(
            x[i].reshape(NT, P, D).transpose(1, 0, 2)).astype(bf)
        in_maps.append({
            "xT": xT_bl,
            "x": x_bl,
            "wvfc": wvfc.reshape(NB, P, 8, 512),
            "id128": id128,
            "grow": grow,
            "brow": brow,
            "crow": crow,
        })
    return in_maps


def kernel(**inputs) -> np.ndarray:
    nc = _get_nc()
    in_maps = _shard_inputs(inputs)
    res = run_bass_kernel_spmd(nc, in_maps, core_ids=list(range(N_CORES)))
    out = np.stack([res.results[i]["out"] for i in range(N_CORES)], axis=0)
    return out.astype(np.float32)


def _install_ntff_hook_shim():
    """Bridge trn_boot's ctypes NTFF profiler into antenv.axon_hooks,
    which bass_utils imports when trace=True under axon."""
    import sys
    import types
    try:
        from antenv.axon_hooks import get_axon_ntff_profile_hook  # noqa: F401
        return
    except ImportError:
        pass
    try:
        from trn_agent_boot.trn_boot import _ntff_profile_via_ctypes
        hook = _ntff_profile_via_ctypes("/opt/axon/libaxon_pjrt.so")
    except Exception:
        hook = None
    mod = types.ModuleType("antenv.axon_hooks")
    state = {"hook": hook}
    mod.get_axon_ntff_profile_hook = lambda: state["hook"]
    mod.set_axon_ntff_profile_hook = lambda h: state.update(hook=h)
    sys.modules["antenv.axon_hooks"] = mod
    import antenv
    antenv.axon_hooks = mod


def kernel_profiled(inputs, trace_cores=None):
    """Like kernel() but with trace=True; returns (out, BassKernelResults)."""
    _install_ntff_hook_shim()
    nc = _get_nc()
    in_maps = _shard_inputs(inputs)
    res = run_bass_kernel_spmd(
        nc, in_maps, core_ids=list(range(N_CORES)), trace=True,
        trace_cores=trace_cores if trace_cores is not None else [0])
    out = np.stack([res.results[i]["out"] for i in range(N_CORES)], axis=0)
    return out.astype(np.float32), res


if __name__ == "__main__":
    import sys
    if "--sim" in sys.argv:
        # quick single-core CoreSim check against the collapsed math
        from concourse.bass_interp import CoreSim
        rng = np.random.default_rng(0)
        x = rng.standard_normal((B, L, D), dtype=np.float32)
        wv = rng.standard_normal((D, HD), dtype=np.float32) * 0.025
        bv = rng.standard_normal(HD, dtype=np.float32) * 0.025
        fc_w = rng.standard_normal((HD, D), dtype=np.float32) * 0.009
        fc_b = rng.standard_normal(D, dtype=np.float32) * 0.015
        g = rng.standard_normal(D, dtype=np.float32) * 0.3 + 1.0
        b = rng.standard_normal(D, dtype=np.float32) * 0.1
        inputs = dict(input=x, wv=wv, bv=bv, fc_w=fc_w, fc_b=fc_b,
                      ln_g=g, ln_b=b)

        nc = _get_nc()
        in_maps = _shard_inputs(inputs)
        sim = CoreSim(nc, trace=False)
        for k, v in in_maps[0].items():
            sim.tensor(k)[:] = v
        sim.simulate()
        got = np.array(sim.tensor("out")).astype(np.float32)

        xsum = x[0].sum(0)
        z = (xsum @ wv + L * bv) @ fc_w + fc_b
        y = x[0] + z[None, :]
        mu = y.mean(-1, keepdims=True)
        var = y.var(-1, keepdims=True)
        want = (y - mu) / np.sqrt(var + EPS) * g + b
        err = np.abs(got - want).max() / np.abs(want).max()
        print("sim absmax rel err:", err)
        assert err < 2e-2, err
        print("SIM PASS")
